# revision 14
# baseline (speedup 1.0000x reference)
"""Causal self-attention (GQA + RMS-norm + RoPE) Trainium2 Bass kernel, v2.

Sharding: 8 cores = 4 batches x 2 head-groups. Core c = 2*b + t handles
batch b with Q heads [8t, 8t+8) and KV heads [2t, 2t+2). Each core computes
a partial output projection; the host upcasts + sums the two partials.

v2 design vs v1 (970us):
- bf16 operands everywhere (host-cast), fp32 PSUM accumulation.
- Fully SBUF-resident: no DRAM scratch spill of qT/kT/v.
- Fused pipeline per 512-token window: QKV production -> RMS+RoPE ->
  PE transpose -> attention; output projection at the end.
- ACT (scalar engine) runs ONLY Exp: no activation-table thrashing.
  RMS rsqrt via quake bit-trick + 2 Newton steps on DVE; softmax
  reciprocal via single-instruction reciprocal_approx_fast on DVE.
"""
import sys
sys.path.insert(0, '/opt/trn_rl_repo')
import numpy as np
import ml_dtypes

from concourse import bass, bacc, mybir, tile

f32 = mybir.dt.float32
f32r = mybir.dt.float32r
bf16 = mybir.dt.bfloat16
i32 = mybir.dt.int32
Alu = mybir.AluOpType
Act = mybir.ActivationFunctionType

B, S, D = 4, 2048, 2048
H, HKV, HD = 16, 4, 128
HLOC = H // 2          # 8 q heads per core
KVLOC = HKV // 2       # 2 kv heads per core
SCALE = float(HD) ** -0.5
ROPE_BASE = 10000.0
MAGIC = 0x5F3759DF
S128 = float(np.sqrt(128.0))

NTC = S // 128         # 16 token tiles
NDT = D // 128         # 16 contraction tiles
NWIN = S // 512        # 4 windows
NH_ALL = HLOC + KVLOC  # 10 rms'd heads per token tile


def _rope_tables():
    inv_freq = (1.0 / (ROPE_BASE ** (np.arange(0, HD, 2, dtype=np.float64) / HD)))
    freqs = np.arange(S, dtype=np.float64)[:, None] * inv_freq[None, :]
    cos = np.cos(freqs)
    sin = np.sin(freqs)
    cos2 = np.concatenate([cos, cos], axis=1).astype(ml_dtypes.bfloat16)
    sin2 = np.concatenate([sin, -sin], axis=1).astype(ml_dtypes.bfloat16)
    return cos2, sin2


def _tri_masks():
    # mask[vi][p, f] = -1e30 where kv > q for scoresT diag tiles:
    # kv = 128*j + p, q = 512*w + f, vi = j - 4*w -> masked iff p + 128*vi > f
    m = np.zeros((4, 128, 512), dtype=np.float32)  # cast to bf16 at the end
    p = np.arange(128)[:, None]
    f = np.arange(512)[None, :]
    for vi in range(4):
        m[vi][(p + 128 * vi) > f] = -1e30
    return m.astype(ml_dtypes.bfloat16)


def _emit_quake_rsqrt(nc, ms_ap, yi_ap, yf_ap, t_ap):
    """yf = sqrt(128) * rsqrt(ms) elementwise on [128, n] fp32 APs.
    ms_ap: input sum-of-squares. yi_ap: int32 scratch. yf_ap/t_ap: f32."""
    nc.vector.tensor_single_scalar(yi_ap, ms_ap.bitcast(i32), 1,
                                   Alu.logical_shift_right)
    nc.vector.tensor_single_scalar(yi_ap, yi_ap, 0xFFFFFFFF, Alu.bitwise_xor)
    nc.vector.tensor_single_scalar(yi_ap, yi_ap, MAGIC + 1, Alu.add)
    nc.vector.tensor_copy(yf_ap, yi_ap.bitcast(f32))
    for last in (False, True):
        nc.vector.tensor_mul(t_ap, yf_ap, yf_ap)
        nc.vector.tensor_mul(t_ap, t_ap, ms_ap)
        s_ = S128 if last else 1.0
        nc.vector.tensor_scalar(t_ap, t_ap, -0.5 * s_, 1.5 * s_,
                                Alu.mult, Alu.add)
        nc.vector.tensor_mul(yf_ap, yf_ap, t_ap)


def build_program():
    cos_np, sin_np = _rope_tables()
    masks_np = _tri_masks()

    nc = bacc.Bacc(trn_type="TRN2")

    xt_d = nc.dram_tensor("xt", [D, S], bf16, kind="ExternalInput")
    wq_d = nc.dram_tensor("wq", [D, HLOC * HD], bf16, kind="ExternalInput")
    wkv_d = nc.dram_tensor("wkv", [D, 2 * KVLOC * HD], bf16, kind="ExternalInput")
    wo_d = nc.dram_tensor("wo", [HLOC * HD, D], bf16, kind="ExternalInput")
    out_d = nc.dram_tensor("out", [S, D], bf16, kind="ExternalOutput")

    cos_d = nc.inline_tensor(cos_np, "cos_t")
    sin_d = nc.inline_tensor(sin_np, "sin_t")
    ident_d = nc.inline_tensor(
        np.eye(128, dtype=np.float32).astype(ml_dtypes.bfloat16), "ident")
    masks_d = nc.inline_tensor(masks_np, "tri_masks")
    onescol_d = nc.inline_tensor(np.ones((128, 1), dtype=ml_dtypes.bfloat16),
                                 "onescol")
    onesrow_d = nc.inline_tensor(np.ones((1, 128), dtype=np.float32), "onesrow")

    with tile.TileContext(nc) as tc:
        with tc.tile_pool(name="cst", bufs=1) as cst:
            cos_sb = cst.tile([128, NTC, 128], bf16, tag="cos")
            sin_sb = cst.tile([128, NTC, 128], bf16, tag="sin")
            ident = cst.tile([128, 128], bf16, tag="ident")
            masks = cst.tile([128, 4, 512], bf16, tag="masks")
            ones = cst.tile([128, 1], bf16, tag="ones")
            ones_r = cst.tile([1, 128], f32r, tag="ones_r")
            nc.sync.dma_start(out=cos_sb[:],
                              in_=cos_d[:].rearrange("(t p) f -> p t f", p=128))
            nc.sync.dma_start(out=sin_sb[:],
                              in_=sin_d[:].rearrange("(t p) f -> p t f", p=128))
            nc.sync.dma_start(out=ident[:], in_=ident_d[:])
            nc.sync.dma_start(out=masks[:],
                              in_=masks_d[:].rearrange("v p f -> p v f"))
            nc.sync.dma_start(out=ones[:], in_=onescol_d[:])
            nc.sync.dma_start(out=ones_r[:], in_=onesrow_d[:].bitcast(f32r))

            # weights resident in bf16
            wq_sb = cst.tile([128, NDT, HLOC * HD], bf16, tag="wq")
            wkv_sb = cst.tile([128, NDT, 512], bf16, tag="wkv")
            wo_sb = cst.tile([128, HLOC, D], bf16, tag="wo")
            wq_r = wq_d[:].rearrange("(t p) c -> p t c", p=128)
            wkv_r = wkv_d[:].rearrange("(t p) c -> p t c", p=128)
            wo_r = wo_d[:].rearrange("(h p) c -> p h c", p=128)
            for dt_ in range(NDT):
                nc.sync.dma_start(out=wkv_sb[:, dt_, :], in_=wkv_r[:, dt_, :])
            for dt_ in range(NDT):
                nc.scalar.dma_start(out=wq_sb[:, dt_, :], in_=wq_r[:, dt_, :])
            for og in range(4):
                nc.scalar.dma_start(out=wo_sb[:, :, og * 512:(og + 1) * 512],
                                    in_=wo_r[:, :, og * 512:(og + 1) * 512])

            # per-window K/V/Q(T) residents + full attention output
            kt_w = [cst.tile([128, KVLOC, 512], bf16, tag=f"kt{w}",
                             name=f"kt{w}")
                    for w in range(NWIN)]
            v_w = [cst.tile([128, 4, 256], bf16, tag=f"v{w}", name=f"v{w}")
                   for w in range(NWIN)]
            ytn = cst.tile([128, HLOC, S], bf16, tag="ytn")

            with tc.tile_pool(name="xs", bufs=2) as xs, \
                 tc.tile_pool(name="qtw", bufs=2) as qtw, \
                 tc.tile_pool(name="nat", bufs=3) as nat, \
                 tc.tile_pool(name="rms", bufs=2) as rms, \
                 tc.tile_pool(name="rop", bufs=2) as rop, \
                 tc.tile_pool(name="ex", bufs=4) as ex, \
                 tc.tile_pool(name="sm", bufs=2) as sm, \
                 tc.tile_pool(name="p1a", bufs=2, space="PSUM") as p1a, \
                 tc.tile_pool(name="p1t", bufs=1, space="PSUM") as p1t, \
                 tc.tile_pool(name="p2s", bufs=3, space="PSUM") as p2s, \
                 tc.tile_pool(name="p2y", bufs=2, space="PSUM") as p2y:

                # transposes of tile i are emitted interleaved into tile
                # (i+1)'s production matmul stream so the PE never stalls on
                # the p1t bank round-trip
                pending_tp = []

                def emit_one_transpose():
                    if pending_tp:
                        src, dst = pending_tp.pop(0)
                        tp = p1t.tile([128, 128], bf16, tag="tp", name="tp")
                        nc.tensor.transpose(tp[:], src, ident[:])
                        nc.vector.tensor_copy(dst, tp[:])

                for w in range(NWIN):
                    qt_sb = qtw.tile([128, HLOC, 512], bf16, tag="qt")

                    # ---------- A(w): QKV production for 4 token tiles ----------
                    for tt in range(4):
                        tcid = 4 * w + tt
                        xt_sb = xs.tile([128, NDT, 128], bf16, tag="xt")
                        nc.sync.dma_start(
                            out=xt_sb[:],
                            in_=xt_d[:, tcid * 128:(tcid + 1) * 128]
                                .rearrange("(t p) s -> p t s", p=128))

                        ps_q1 = p1a.tile([128, 512], f32, tag="acc")
                        ps_q2 = p1a.tile([128, 512], f32, tag="acc")
                        ps_kv = p1a.tile([128, 512], f32, tag="acc")
                        for dt_ in range(NDT):
                            st, sp = dt_ == 0, dt_ == NDT - 1
                            lhs = xt_sb[:, dt_, :]
                            nc.tensor.matmul(ps_q1[:], lhs,
                                             wq_sb[:, dt_, 0:512],
                                             start=st, stop=sp)
                            nc.tensor.matmul(ps_q2[:], lhs,
                                             wq_sb[:, dt_, 512:1024],
                                             start=st, stop=sp)
                            nc.tensor.matmul(ps_kv[:], lhs,
                                             wkv_sb[:, dt_, :],
                                             start=st, stop=sp)
                            emit_one_transpose()

                        cos1 = cos_sb[:, tcid:tcid + 1, :]
                        sin1 = sin_sb[:, tcid:tcid + 1, :]

                        # RMS stats (ACT square, DVE reduce) + rope muls,
                        # ordered for early PSUM release
                        ms = rms.tile([128, NH_ALL, 1], f32, tag="ms")
                        groups = ((ps_q1, 4, 0), (ps_q2, 4, 4), (ps_kv, 2, 8))
                        t1s = []
                        sqs = []
                        for gi, (ps, nh, g0) in enumerate(groups):
                            wdt = nh * 128
                            shp = [128, nh, 2, 64]
                            p4 = ps[:, 0:wdt].rearrange(
                                "p (h x f) -> p h x f", h=nh, x=2)
                            p4s = p4[:, :, ::-1, :]
                            cb = cos1.rearrange("p t (x f) -> p t x f", x=2) \
                                     .to_broadcast(shp)
                            sb_ = sin1.rearrange("p t (x f) -> p t x f", x=2) \
                                      .to_broadcast(shp)
                            sq = rop.tile([128, 512], bf16, tag="sq", bufs=2)
                            t1 = rop.tile([128, 4, 2, 64], f32, tag="t1",
                                          bufs=3)
                            t2 = rop.tile([128, 4, 2, 64], f32, tag="t2",
                                          bufs=2)
                            nc.scalar.activation(sq[:, 0:wdt], ps[:, 0:wdt],
                                                 Act.Square)
                            nc.vector.tensor_mul(t1[:, 0:nh], p4, cb)
                            nc.vector.tensor_mul(t2[:, 0:nh], p4s, sb_)
                            nc.vector.tensor_add(t1[:, 0:nh], t1[:, 0:nh],
                                                 t2[:, 0:nh])
                            t1s.append((t1, nh, g0))
                            sqs.append((sq, nh, g0))
                        # v copy out of kv psum (ACT)
                        nc.scalar.activation(v_w[w][:, tt, :],
                                             ps_kv[:, 256:512], Act.Copy)

                        for sq, nh, g0 in sqs:
                            nc.vector.tensor_reduce(
                                ms[:, g0:g0 + nh, :],
                                sq[:, 0:nh * 128].rearrange(
                                    "p (h f) -> p h f", h=nh),
                                axis=mybir.AxisListType.X, op=Alu.add)

                        yi = rms.tile([128, NH_ALL], i32, tag="yi")
                        rinv = rms.tile([128, NH_ALL, 1, 1], f32, tag="rinv")
                        tq = rms.tile([128, NH_ALL], f32, tag="tq")
                        _emit_quake_rsqrt(
                            nc, ms[:].rearrange("p h f -> p (h f)"), yi[:],
                            rinv[:].rearrange("p h x f -> p (h x f)"), tq[:])

                        qn = nat.tile([128, 1024], bf16, tag="qn")
                        kn = nat.tile([128, 256], bf16, tag="kn")
                        outs = (qn[:, 0:512], qn[:, 512:1024], kn[:, 0:256])
                        for (t1, nh, g0), o in zip(t1s, outs):
                            shp = [128, nh, 2, 64]
                            rb = rinv[:, g0:g0 + nh].to_broadcast(shp)
                            nc.vector.tensor_mul(
                                o.rearrange("p (h x f) -> p h x f", h=nh, x=2),
                                t1[:, 0:nh], rb)

                        for h in range(HLOC):
                            pending_tp.append(
                                (qn[:, h * 128:(h + 1) * 128],
                                 qt_sb[:, h, tt * 128:(tt + 1) * 128]))
                        for kh in range(KVLOC):
                            pending_tp.append(
                                (kn[:, kh * 128:(kh + 1) * 128],
                                 kt_w[w][:, kh, tt * 128:(tt + 1) * 128]))

                    # flush the last tile's transposes before attention
                    while pending_tp:
                        emit_one_transpose()

                    # ---------- B(w): attention for this query window ----------
                    njt = 4 * w + 4
                    for hq in range(HLOC):
                        kvh = hq // 4
                        ps_y = p2y.tile([128, 512], f32, tag="y")
                        s_acc = sm.tile([1, 512], f32, tag="sacc")
                        rhs_q = qt_sb[:, hq, :]
                        for j in range(njt):
                            jw, jj = j // 4, j % 4
                            ps_sc = p2s.tile([128, 512], f32, tag="sc")
                            nc.tensor.matmul(
                                ps_sc[:],
                                kt_w[jw][:, kvh, jj * 128:(jj + 1) * 128],
                                rhs_q)
                            if j >= 4 * w:
                                nc.vector.tensor_add(ps_sc[:], ps_sc[:],
                                                     masks[:, j - 4 * w, :])
                            et = ex.tile([128, 512], bf16, tag="et")
                            nc.scalar.activation(et[:], ps_sc[:], Act.Exp,
                                                 scale=SCALE)
                            st, sp = j == 0, j == njt - 1
                            nc.tensor.matmul(
                                ps_y[:],
                                v_w[jw][:, jj, kvh * 128:(kvh + 1) * 128],
                                et[:], start=st, stop=sp,
                                skip_group_check=True)
                            # softmax denominator on the (otherwise idle)
                            # GPSIMD engine: partition-axis reduce + accumulate
                            if j == 0:
                                nc.gpsimd.tensor_reduce(
                                    s_acc[:], et[:],
                                    axis=mybir.AxisListType.C, op=Alu.add)
                            else:
                                srow = sm.tile([1, 512], f32, tag="srow",
                                               bufs=2)
                                nc.gpsimd.tensor_reduce(
                                    srow[:], et[:],
                                    axis=mybir.AxisListType.C, op=Alu.add)
                                nc.gpsimd.tensor_add(s_acc[:], s_acc[:],
                                                     srow[:])
                        rec = sm.tile([1, 512], f32, tag="rec")
                        nc.vector.reciprocal_approx_fast(out=rec[:],
                                                         in_=s_acc[:])
                        rec_r = sm.tile([1, 512], f32r, tag="rec_r")
                        nc.vector.tensor_copy(rec_r[:], rec[:])
                        bcp = p2s.tile([128, 512], f32, tag="sc")
                        nc.tensor.matmul(bcp[:], ones_r[:], rec_r[:])
                        bc = sm.tile([128, 512], bf16, tag="bc")
                        nc.scalar.activation(bc[:], bcp[:], Act.Copy)
                        nc.vector.tensor_mul(
                            ytn[:, hq, w * 512:(w + 1) * 512], ps_y[:], bc[:])

            # ---------- C: output projection ----------
            with tc.tile_pool(name="ob", bufs=4) as ob, \
                 tc.tile_pool(name="p3", bufs=4, space="PSUM") as p3:
                for og in range(4):
                    for tcid in range(NTC):
                        ps_o = p3.tile([128, 512], f32, tag="o")
                        for h in range(HLOC):
                            nc.tensor.matmul(
                                ps_o[:],
                                ytn[:, h, tcid * 128:(tcid + 1) * 128],
                                wo_sb[:, h, og * 512:(og + 1) * 512],
                                start=(h == 0), stop=(h == HLOC - 1))
                        ot = ob.tile([128, 512], bf16, tag="ot")
                        nc.scalar.activation(ot[:], ps_o[:], Act.Copy)
                        nc.scalar.dma_start(
                            out=out_d[tcid * 128:(tcid + 1) * 128,
                                      og * 512:(og + 1) * 512],
                            in_=ot[:])

    nc.compile()
    return nc


_PROGRAM = None


def _get_program():
    global _PROGRAM
    if _PROGRAM is None:
        _PROGRAM = build_program()
    return _PROGRAM


def make_in_maps(x, W_qkv, W_out):
    bf = ml_dtypes.bfloat16
    in_maps = []
    for c in range(8):
        b, t = c // 2, c % 2
        xt = np.ascontiguousarray(x[b].T).astype(bf)
        wq = np.ascontiguousarray(W_qkv[:, t * 1024:(t + 1) * 1024]).astype(bf)
        wk = W_qkv[:, D + t * 256: D + (t + 1) * 256]
        wv = W_qkv[:, D + 512 + t * 256: D + 512 + (t + 1) * 256]
        wkv = np.ascontiguousarray(np.concatenate([wk, wv], axis=1)).astype(bf)
        wo = np.ascontiguousarray(W_out[t * 1024:(t + 1) * 1024, :]).astype(bf)
        in_maps.append({"xt": xt, "wq": wq, "wkv": wkv, "wo": wo})
    return in_maps


def kernel(x, W_qkv, W_out):
    from concourse.bass_utils import run_bass_kernel_spmd
    nc = _get_program()
    in_maps = make_in_maps(np.asarray(x, dtype=np.float32),
                           np.asarray(W_qkv, dtype=np.float32),
                           np.asarray(W_out, dtype=np.float32))
    res = run_bass_kernel_spmd(nc, in_maps, list(range(8)), trace=False)
    out = np.empty((B, S, D), dtype=np.float32)
    for b in range(B):
        out[b] = (res.results[2 * b]["out"].astype(np.float32)
                  + res.results[2 * b + 1]["out"].astype(np.float32))
    return out


# revision 15
# speedup vs baseline: 27.8438x; 27.8438x over previous
"""Causal self-attention (GQA + RMS-norm + RoPE) Trainium2 Bass kernel, v2.

Sharding: 8 cores = 4 batches x 2 head-groups. Core c = 2*b + t handles
batch b with Q heads [8t, 8t+8) and KV heads [2t, 2t+2). Each core computes
a partial output projection; the host upcasts + sums the two partials.

v2 design vs v1 (970us):
- bf16 operands everywhere (host-cast), fp32 PSUM accumulation.
- Fully SBUF-resident: no DRAM scratch spill of qT/kT/v.
- Fused pipeline per 512-token window: QKV production -> RMS+RoPE ->
  PE transpose -> attention; output projection at the end.
- ACT (scalar engine) runs ONLY Exp: no activation-table thrashing.
  RMS rsqrt via quake bit-trick + 2 Newton steps on DVE; softmax
  reciprocal via single-instruction reciprocal_approx_fast on DVE.
"""
import sys
sys.path.insert(0, '/opt/trn_rl_repo')
import numpy as np
import ml_dtypes

from concourse import bass, bacc, mybir, tile

f32 = mybir.dt.float32
f32r = mybir.dt.float32r
bf16 = mybir.dt.bfloat16
i32 = mybir.dt.int32
Alu = mybir.AluOpType
Act = mybir.ActivationFunctionType

B, S, D = 4, 2048, 2048
H, HKV, HD = 16, 4, 128
HLOC = H // 2          # 8 q heads per core
KVLOC = HKV // 2       # 2 kv heads per core
SCALE = float(HD) ** -0.5
ROPE_BASE = 10000.0
MAGIC = 0x5F3759DF
S128 = float(np.sqrt(128.0))

NTC = S // 128         # 16 token tiles
NDT = D // 128         # 16 contraction tiles
NWIN = S // 512        # 4 windows
NH_ALL = HLOC + KVLOC  # 10 rms'd heads per token tile


def _rope_tables():
    inv_freq = (1.0 / (ROPE_BASE ** (np.arange(0, HD, 2, dtype=np.float64) / HD)))
    freqs = np.arange(S, dtype=np.float64)[:, None] * inv_freq[None, :]
    cos = np.cos(freqs)
    sin = np.sin(freqs)
    cos2 = np.concatenate([cos, cos], axis=1).astype(ml_dtypes.bfloat16)
    sin2 = np.concatenate([sin, -sin], axis=1).astype(ml_dtypes.bfloat16)
    return cos2, sin2


def _tri_masks():
    # mask[vi][p, f] = -1e30 where kv > q for scoresT diag tiles:
    # kv = 128*j + p, q = 512*w + f, vi = j - 4*w -> masked iff p + 128*vi > f
    m = np.zeros((4, 128, 512), dtype=np.float32)  # cast to bf16 at the end
    p = np.arange(128)[:, None]
    f = np.arange(512)[None, :]
    for vi in range(4):
        m[vi][(p + 128 * vi) > f] = -1e30
    return m.astype(ml_dtypes.bfloat16)


def _emit_quake_rsqrt(nc, ms_ap, yi_ap, yf_ap, t_ap):
    """yf = sqrt(128) * rsqrt(ms) elementwise on [128, n] fp32 APs.
    ms_ap: input sum-of-squares. yi_ap: int32 scratch. yf_ap/t_ap: f32."""
    nc.vector.tensor_single_scalar(yi_ap, ms_ap.bitcast(i32), 1,
                                   Alu.logical_shift_right)
    nc.vector.tensor_single_scalar(yi_ap, yi_ap, 0xFFFFFFFF, Alu.bitwise_xor)
    nc.vector.tensor_single_scalar(yi_ap, yi_ap, MAGIC + 1, Alu.add)
    nc.vector.tensor_copy(yf_ap, yi_ap.bitcast(f32))
    for last in (False, True):
        nc.vector.tensor_mul(t_ap, yf_ap, yf_ap)
        nc.vector.tensor_mul(t_ap, t_ap, ms_ap)
        s_ = S128 if last else 1.0
        nc.vector.tensor_scalar(t_ap, t_ap, -0.5 * s_, 1.5 * s_,
                                Alu.mult, Alu.add)
        nc.vector.tensor_mul(yf_ap, yf_ap, t_ap)


def build_program():
    cos_np, sin_np = _rope_tables()
    masks_np = _tri_masks()

    nc = bacc.Bacc(trn_type="TRN2")

    xt_d = nc.dram_tensor("xt", [D, S], bf16, kind="ExternalInput")
    wq_d = nc.dram_tensor("wq", [D, HLOC * HD], bf16, kind="ExternalInput")
    wkv_d = nc.dram_tensor("wkv", [D, 2 * KVLOC * HD], bf16, kind="ExternalInput")
    wo_d = nc.dram_tensor("wo", [HLOC * HD, D], bf16, kind="ExternalInput")
    out_d = nc.dram_tensor("out", [S, D], bf16, kind="ExternalOutput")

    cos_d = nc.inline_tensor(cos_np, "cos_t")
    sin_d = nc.inline_tensor(sin_np, "sin_t")
    ident_d = nc.inline_tensor(
        np.eye(128, dtype=np.float32).astype(ml_dtypes.bfloat16), "ident")
    masks_d = nc.inline_tensor(masks_np, "tri_masks")
    onescol_d = nc.inline_tensor(np.ones((128, 1), dtype=ml_dtypes.bfloat16),
                                 "onescol")
    onesrow_d = nc.inline_tensor(np.ones((1, 128), dtype=np.float32), "onesrow")

    with tile.TileContext(nc) as tc:
        with tc.tile_pool(name="cst", bufs=1) as cst:
            cos_sb = cst.tile([128, NTC, 128], bf16, tag="cos")
            sin_sb = cst.tile([128, NTC, 128], bf16, tag="sin")
            ident = cst.tile([128, 128], bf16, tag="ident")
            masks = cst.tile([128, 4, 512], bf16, tag="masks")
            ones = cst.tile([128, 1], bf16, tag="ones")
            ones_r = cst.tile([1, 128], f32r, tag="ones_r")
            nc.sync.dma_start(out=cos_sb[:],
                              in_=cos_d[:].rearrange("(t p) f -> p t f", p=128))
            nc.sync.dma_start(out=sin_sb[:],
                              in_=sin_d[:].rearrange("(t p) f -> p t f", p=128))
            nc.sync.dma_start(out=ident[:], in_=ident_d[:])
            nc.sync.dma_start(out=masks[:],
                              in_=masks_d[:].rearrange("v p f -> p v f"))
            nc.sync.dma_start(out=ones[:], in_=onescol_d[:])
            nc.sync.dma_start(out=ones_r[:], in_=onesrow_d[:].bitcast(f32r))

            # weights resident in bf16
            wq_sb = cst.tile([128, NDT, HLOC * HD], bf16, tag="wq")
            wkv_sb = cst.tile([128, NDT, 512], bf16, tag="wkv")
            wo_sb = cst.tile([128, HLOC, D], bf16, tag="wo")
            wq_r = wq_d[:].rearrange("(t p) c -> p t c", p=128)
            wkv_r = wkv_d[:].rearrange("(t p) c -> p t c", p=128)
            wo_r = wo_d[:].rearrange("(h p) c -> p h c", p=128)
            for dt_ in range(NDT):
                nc.sync.dma_start(out=wkv_sb[:, dt_, :], in_=wkv_r[:, dt_, :])
            for dt_ in range(NDT):
                nc.scalar.dma_start(out=wq_sb[:, dt_, :], in_=wq_r[:, dt_, :])
            for og in range(4):
                nc.scalar.dma_start(out=wo_sb[:, :, og * 512:(og + 1) * 512],
                                    in_=wo_r[:, :, og * 512:(og + 1) * 512])

            # per-window K/V/Q(T) residents + full attention output
            kt_w = [cst.tile([128, KVLOC, 512], bf16, tag=f"kt{w}",
                             name=f"kt{w}")
                    for w in range(NWIN)]
            v_w = [cst.tile([128, 4, 256], bf16, tag=f"v{w}", name=f"v{w}")
                   for w in range(NWIN)]
            ytn = cst.tile([128, HLOC, S], bf16, tag="ytn")

            with tc.tile_pool(name="xs", bufs=2) as xs, \
                 tc.tile_pool(name="qtw", bufs=2) as qtw, \
                 tc.tile_pool(name="nat", bufs=3) as nat, \
                 tc.tile_pool(name="rms", bufs=2) as rms, \
                 tc.tile_pool(name="rop", bufs=2) as rop, \
                 tc.tile_pool(name="ex", bufs=4) as ex, \
                 tc.tile_pool(name="sm", bufs=2) as sm, \
                 tc.tile_pool(name="p1a", bufs=2, space="PSUM") as p1a, \
                 tc.tile_pool(name="p1t", bufs=1, space="PSUM") as p1t, \
                 tc.tile_pool(name="p2s", bufs=3, space="PSUM") as p2s, \
                 tc.tile_pool(name="p2y", bufs=2, space="PSUM") as p2y:

                # transposes of tile i are emitted interleaved into tile
                # (i+1)'s production matmul stream so the PE never stalls on
                # the p1t bank round-trip
                pending_tp = []

                def emit_one_transpose():
                    if pending_tp:
                        src, dst = pending_tp.pop(0)
                        tp = p1t.tile([128, 128], bf16, tag="tp", name="tp")
                        nc.tensor.transpose(tp[:], src, ident[:])
                        nc.vector.tensor_copy(dst, tp[:])

                for w in range(NWIN):
                    qt_sb = qtw.tile([128, HLOC, 512], bf16, tag="qt")

                    # ---------- A(w): QKV production for 4 token tiles ----------
                    for tt in range(4):
                        tcid = 4 * w + tt
                        xt_sb = xs.tile([128, NDT, 128], bf16, tag="xt")
                        nc.sync.dma_start(
                            out=xt_sb[:],
                            in_=xt_d[:, tcid * 128:(tcid + 1) * 128]
                                .rearrange("(t p) s -> p t s", p=128))

                        ps_q1 = p1a.tile([128, 512], f32, tag="acc")
                        ps_q2 = p1a.tile([128, 512], f32, tag="acc")
                        ps_kv = p1a.tile([128, 512], f32, tag="acc")
                        for dt_ in range(NDT):
                            st, sp = dt_ == 0, dt_ == NDT - 1
                            lhs = xt_sb[:, dt_, :]
                            nc.tensor.matmul(ps_q1[:], lhs,
                                             wq_sb[:, dt_, 0:512],
                                             start=st, stop=sp)
                            nc.tensor.matmul(ps_q2[:], lhs,
                                             wq_sb[:, dt_, 512:1024],
                                             start=st, stop=sp)
                            nc.tensor.matmul(ps_kv[:], lhs,
                                             wkv_sb[:, dt_, :],
                                             start=st, stop=sp)
                            emit_one_transpose()

                        cos1 = cos_sb[:, tcid:tcid + 1, :]
                        sin1 = sin_sb[:, tcid:tcid + 1, :]

                        # RMS stats (ACT square, DVE reduce) + rope muls,
                        # ordered for early PSUM release
                        ms = rms.tile([128, NH_ALL, 1], f32, tag="ms")
                        groups = ((ps_q1, 4, 0), (ps_q2, 4, 4), (ps_kv, 2, 8))
                        t1s = []
                        sqs = []
                        for gi, (ps, nh, g0) in enumerate(groups):
                            wdt = nh * 128
                            shp = [128, nh, 2, 64]
                            p4 = ps[:, 0:wdt].rearrange(
                                "p (h x f) -> p h x f", h=nh, x=2)
                            p4s = p4[:, :, ::-1, :]
                            cb = cos1.rearrange("p t (x f) -> p t x f", x=2) \
                                     .to_broadcast(shp)
                            sb_ = sin1.rearrange("p t (x f) -> p t x f", x=2) \
                                      .to_broadcast(shp)
                            sq = rop.tile([128, 512], bf16, tag="sq", bufs=2)
                            t1 = rop.tile([128, 4, 2, 64], f32, tag="t1",
                                          bufs=3)
                            t2 = rop.tile([128, 4, 2, 64], f32, tag="t2",
                                          bufs=2)
                            nc.scalar.activation(sq[:, 0:wdt], ps[:, 0:wdt],
                                                 Act.Square)
                            nc.vector.tensor_mul(t1[:, 0:nh], p4, cb)
                            nc.vector.tensor_mul(t2[:, 0:nh], p4s, sb_)
                            nc.vector.tensor_add(t1[:, 0:nh], t1[:, 0:nh],
                                                 t2[:, 0:nh])
                            t1s.append((t1, nh, g0))
                            sqs.append((sq, nh, g0))
                        # v copy out of kv psum (ACT)
                        nc.scalar.activation(v_w[w][:, tt, :],
                                             ps_kv[:, 256:512], Act.Copy)

                        for sq, nh, g0 in sqs:
                            nc.vector.tensor_reduce(
                                ms[:, g0:g0 + nh, :],
                                sq[:, 0:nh * 128].rearrange(
                                    "p (h f) -> p h f", h=nh),
                                axis=mybir.AxisListType.X, op=Alu.add)

                        yi = rms.tile([128, NH_ALL], i32, tag="yi")
                        rinv = rms.tile([128, NH_ALL, 1, 1], f32, tag="rinv")
                        tq = rms.tile([128, NH_ALL], f32, tag="tq")
                        _emit_quake_rsqrt(
                            nc, ms[:].rearrange("p h f -> p (h f)"), yi[:],
                            rinv[:].rearrange("p h x f -> p (h x f)"), tq[:])

                        qn = nat.tile([128, 1024], bf16, tag="qn")
                        kn = nat.tile([128, 256], bf16, tag="kn")
                        outs = (qn[:, 0:512], qn[:, 512:1024], kn[:, 0:256])
                        for (t1, nh, g0), o in zip(t1s, outs):
                            shp = [128, nh, 2, 64]
                            rb = rinv[:, g0:g0 + nh].to_broadcast(shp)
                            nc.vector.tensor_mul(
                                o.rearrange("p (h x f) -> p h x f", h=nh, x=2),
                                t1[:, 0:nh], rb)

                        for h in range(HLOC):
                            pending_tp.append(
                                (qn[:, h * 128:(h + 1) * 128],
                                 qt_sb[:, h, tt * 128:(tt + 1) * 128]))
                        for kh in range(KVLOC):
                            pending_tp.append(
                                (kn[:, kh * 128:(kh + 1) * 128],
                                 kt_w[w][:, kh, tt * 128:(tt + 1) * 128]))

                    # flush the last tile's transposes before attention
                    while pending_tp:
                        emit_one_transpose()

                    # ---------- B(w): attention for this query window ----------
                    njt = 4 * w + 4
                    for hq in range(HLOC):
                        kvh = hq // 4
                        ps_y = p2y.tile([128, 512], f32, tag="y")
                        ps_s = p2s.tile([1, 512], f32, tag="sc")
                        rhs_q = qt_sb[:, hq, :]
                        for j in range(njt):
                            jw, jj = j // 4, j % 4
                            ps_sc = p2s.tile([128, 512], f32, tag="sc")
                            nc.tensor.matmul(
                                ps_sc[:],
                                kt_w[jw][:, kvh, jj * 128:(jj + 1) * 128],
                                rhs_q)
                            if j >= 4 * w:
                                nc.vector.tensor_add(ps_sc[:], ps_sc[:],
                                                     masks[:, j - 4 * w, :])
                            et = ex.tile([128, 512], bf16, tag="et")
                            nc.scalar.activation(et[:], ps_sc[:], Act.Exp,
                                                 scale=SCALE)
                            st, sp = j == 0, j == njt - 1
                            nc.tensor.matmul(
                                ps_y[:],
                                v_w[jw][:, jj, kvh * 128:(kvh + 1) * 128],
                                et[:], start=st, stop=sp,
                                skip_group_check=True)
                            nc.tensor.matmul(
                                ps_s[:], ones[:], et[:],
                                start=st, stop=sp, skip_group_check=True)
                        rec = sm.tile([1, 512], f32, tag="rec")
                        nc.vector.reciprocal_approx_fast(out=rec[:],
                                                         in_=ps_s[:])
                        rec_r = sm.tile([1, 512], f32r, tag="rec_r")
                        nc.vector.tensor_copy(rec_r[:], rec[:])
                        bcp = p2s.tile([128, 512], f32, tag="sc")
                        nc.tensor.matmul(bcp[:], ones_r[:], rec_r[:])
                        bc = sm.tile([128, 512], bf16, tag="bc")
                        nc.scalar.activation(bc[:], bcp[:], Act.Copy)
                        nc.vector.tensor_mul(
                            ytn[:, hq, w * 512:(w + 1) * 512], ps_y[:], bc[:])

            # ---------- C: output projection ----------
            with tc.tile_pool(name="ob", bufs=4) as ob, \
                 tc.tile_pool(name="p3", bufs=4, space="PSUM") as p3:
                for og in range(4):
                    for tcid in range(NTC):
                        ps_o = p3.tile([128, 512], f32, tag="o")
                        for h in range(HLOC):
                            nc.tensor.matmul(
                                ps_o[:],
                                ytn[:, h, tcid * 128:(tcid + 1) * 128],
                                wo_sb[:, h, og * 512:(og + 1) * 512],
                                start=(h == 0), stop=(h == HLOC - 1))
                        ot = ob.tile([128, 512], bf16, tag="ot")
                        nc.scalar.activation(ot[:], ps_o[:], Act.Copy)
                        nc.scalar.dma_start(
                            out=out_d[tcid * 128:(tcid + 1) * 128,
                                      og * 512:(og + 1) * 512],
                            in_=ot[:])

    nc.compile()
    return nc


_PROGRAM = None


def _get_program():
    global _PROGRAM
    if _PROGRAM is None:
        _PROGRAM = build_program()
    return _PROGRAM


def make_in_maps(x, W_qkv, W_out):
    bf = ml_dtypes.bfloat16
    in_maps = []
    for c in range(8):
        b, t = c // 2, c % 2
        xt = np.ascontiguousarray(x[b].T).astype(bf)
        wq = np.ascontiguousarray(W_qkv[:, t * 1024:(t + 1) * 1024]).astype(bf)
        wk = W_qkv[:, D + t * 256: D + (t + 1) * 256]
        wv = W_qkv[:, D + 512 + t * 256: D + 512 + (t + 1) * 256]
        wkv = np.ascontiguousarray(np.concatenate([wk, wv], axis=1)).astype(bf)
        wo = np.ascontiguousarray(W_out[t * 1024:(t + 1) * 1024, :]).astype(bf)
        in_maps.append({"xt": xt, "wq": wq, "wkv": wkv, "wo": wo})
    return in_maps


def kernel(x, W_qkv, W_out):
    from concourse.bass_utils import run_bass_kernel_spmd
    nc = _get_program()
    in_maps = make_in_maps(np.asarray(x, dtype=np.float32),
                           np.asarray(W_qkv, dtype=np.float32),
                           np.asarray(W_out, dtype=np.float32))
    res = run_bass_kernel_spmd(nc, in_maps, list(range(8)), trace=False)
    out = np.empty((B, S, D), dtype=np.float32)
    for b in range(B):
        out[b] = (res.results[2 * b]["out"].astype(np.float32)
                  + res.results[2 * b + 1]["out"].astype(np.float32))
    return out


# revision 18
# speedup vs baseline: 32.6968x; 1.1743x over previous
"""Causal self-attention (GQA + RMS-norm + RoPE) Trainium2 Bass kernel, v5.

Sharding: 8 cores = 4 batches x 2 head-groups. Core c = 2*b + t handles
batch b with Q heads [8t, 8t+8) and KV heads [2t, 2t+2). Each core computes
a partial output projection; the host upcasts + sums the two partials.

Design:
- bf16 operands everywhere (host-cast), fp32 PSUM accumulation.
- Fully SBUF-resident: no DRAM scratch spill of qT/kT/v.
- ACT runs only {Exp, Square, Copy} (one table set: zero table loads).
  RMS rsqrt via quake bit-trick + 2 Newton steps on DVE; softmax
  reciprocal via single-instruction reciprocal_approx_fast on DVE.
- Host pre-tiles x / cos / sin / masks so every DMA line is >=4KB
  per partition.
- Emission interleaves independent PE work into the attention j-loops
  (QKV production of window w+1, out-projection of finished windows)
  so the in-order PE queue never starves on mask/exp latency; per-head
  softmax-normalize tails are deferred one head for the same reason.
"""
import sys
sys.path.insert(0, '/opt/trn_rl_repo')
import numpy as np
import ml_dtypes

from concourse import bass, bacc, mybir, tile

f32 = mybir.dt.float32
f32r = mybir.dt.float32r
bf16 = mybir.dt.bfloat16
i32 = mybir.dt.int32
Alu = mybir.AluOpType
Act = mybir.ActivationFunctionType

B, S, D = 4, 2048, 2048
H, HKV, HD = 16, 4, 128
HLOC = H // 2          # 8 q heads per core
KVLOC = HKV // 2       # 2 kv heads per core
SCALE = float(HD) ** -0.5
ROPE_BASE = 10000.0
MAGIC = 0x5F3759DF
S128 = float(np.sqrt(128.0))

NTC = S // 128         # 16 token tiles
NDT = D // 128         # 16 contraction tiles
NWIN = S // 512        # 4 windows
NH_ALL = HLOC + KVLOC  # 10 rms'd heads per token tile


def _rope_tables():
    inv_freq = (1.0 / (ROPE_BASE ** (np.arange(0, HD, 2, dtype=np.float64) / HD)))
    freqs = np.arange(S, dtype=np.float64)[:, None] * inv_freq[None, :]
    cos = np.cos(freqs)
    sin = np.sin(freqs)
    cos2 = np.concatenate([cos, cos], axis=1).astype(ml_dtypes.bfloat16)
    sin2 = np.concatenate([sin, -sin], axis=1).astype(ml_dtypes.bfloat16)
    # partition-major pre-tiling: [p, tcid, f] with p = token % 128
    cosA = np.ascontiguousarray(cos2.reshape(NTC, 128, HD).transpose(1, 0, 2))
    sinA = np.ascontiguousarray(sin2.reshape(NTC, 128, HD).transpose(1, 0, 2))
    return cosA, sinA


def _tri_masks():
    # mask[p, vi, f] = -1e30 where kv > q for scoresT diag tiles:
    # kv = 128*j + p, q = 512*w + f, vi = j - 4*w -> masked iff p + 128*vi > f
    m = np.zeros((4, 128, 512), dtype=np.float32)
    p = np.arange(128)[:, None]
    f = np.arange(512)[None, :]
    for vi in range(4):
        m[vi][(p + 128 * vi) > f] = -1e30
    return np.ascontiguousarray(
        m.astype(ml_dtypes.bfloat16).transpose(1, 0, 2))


def _emit_quake_rsqrt(nc, ms_ap, yi_ap, yf_ap, t_ap):
    """yf = sqrt(128) * rsqrt(ms) elementwise on [128, n] fp32 APs."""
    nc.vector.tensor_single_scalar(yi_ap, ms_ap.bitcast(i32), 1,
                                   Alu.logical_shift_right)
    nc.vector.tensor_single_scalar(yi_ap, yi_ap, 0xFFFFFFFF, Alu.bitwise_xor)
    nc.vector.tensor_single_scalar(yi_ap, yi_ap, MAGIC + 1, Alu.add)
    nc.vector.tensor_copy(yf_ap, yi_ap.bitcast(f32))
    for last in (False, True):
        nc.vector.tensor_mul(t_ap, yf_ap, yf_ap)
        nc.vector.tensor_mul(t_ap, t_ap, ms_ap)
        s_ = S128 if last else 1.0
        nc.vector.tensor_scalar(t_ap, t_ap, -0.5 * s_, 1.5 * s_,
                                Alu.mult, Alu.add)
        nc.vector.tensor_mul(yf_ap, yf_ap, t_ap)


def build_program():
    cos_np, sin_np = _rope_tables()
    masks_np = _tri_masks()

    nc = bacc.Bacc(trn_type="TRN2")

    # xt pre-tiled on host: xt[tc, p, dt*128+s] = x.T[dt*128+p, tc*128+s]
    xt_d = nc.dram_tensor("xt", [NTC, 128, D], bf16, kind="ExternalInput")
    wq_d = nc.dram_tensor("wq", [D, HLOC * HD], bf16, kind="ExternalInput")
    wkv_d = nc.dram_tensor("wkv", [D, 2 * KVLOC * HD], bf16, kind="ExternalInput")
    wo_d = nc.dram_tensor("wo", [HLOC * HD, D], bf16, kind="ExternalInput")
    out_d = nc.dram_tensor("out", [S, D], bf16, kind="ExternalOutput")

    cos_d = nc.inline_tensor(cos_np, "cos_t")
    sin_d = nc.inline_tensor(sin_np, "sin_t")
    ident_d = nc.inline_tensor(
        np.eye(128, dtype=np.float32).astype(ml_dtypes.bfloat16), "ident")
    masks_d = nc.inline_tensor(masks_np, "tri_masks")
    onescol_d = nc.inline_tensor(np.ones((128, 1), dtype=ml_dtypes.bfloat16),
                                 "onescol")
    onesrow_d = nc.inline_tensor(np.ones((1, 128), dtype=np.float32), "onesrow")

    with tile.TileContext(nc) as tc:
        with tc.tile_pool(name="cst", bufs=1) as cst:
            # tiny constants first (sync queue)
            ident = cst.tile([128, 128], bf16, tag="ident")
            ones = cst.tile([128, 1], bf16, tag="ones")
            ones_r = cst.tile([1, 128], f32r, tag="ones_r")
            nc.sync.dma_start(out=ident[:], in_=ident_d[:])
            nc.sync.dma_start(out=ones[:], in_=onescol_d[:])
            nc.sync.dma_start(out=ones_r[:], in_=onesrow_d[:].bitcast(f32r))

            # bulk constants on the vector DMA queue (off the x-load path)
            cos_sb = cst.tile([128, NTC, 128], bf16, tag="cos")
            sin_sb = cst.tile([128, NTC, 128], bf16, tag="sin")
            masks = cst.tile([128, 4, 512], bf16, tag="masks")
            nc.gpsimd.dma_start(out=cos_sb[:], in_=cos_d[:])
            nc.gpsimd.dma_start(out=sin_sb[:], in_=sin_d[:])
            nc.gpsimd.dma_start(out=masks[:], in_=masks_d[:])

            # weights: wq/wkv needed first (scalar + vector queues); wo last
            wq_sb = cst.tile([128, NDT, HLOC * HD], bf16, tag="wq")
            wkv_sb = cst.tile([128, NDT, 512], bf16, tag="wkv")
            wo_sb = cst.tile([128, HLOC, D], bf16, tag="wo")
            wq_r = wq_d[:].rearrange("(t p) c -> p t c", p=128)
            wkv_r = wkv_d[:].rearrange("(t p) c -> p t c", p=128)
            wo_r = wo_d[:].rearrange("(h p) c -> p h c", p=128)
            for dt_ in range(NDT):
                nc.scalar.dma_start(out=wq_sb[:, dt_, :], in_=wq_r[:, dt_, :])
                nc.gpsimd.dma_start(out=wkv_sb[:, dt_, :], in_=wkv_r[:, dt_, :])
            for og in range(4):
                nc.scalar.dma_start(out=wo_sb[:, :, og * 512:(og + 1) * 512],
                                    in_=wo_r[:, :, og * 512:(og + 1) * 512])

            # per-window K/V residents + full attention output
            kt_w = [cst.tile([128, KVLOC, 512], bf16, tag=f"kt{w}",
                             name=f"kt{w}")
                    for w in range(NWIN)]
            v_w = [cst.tile([128, 4, 256], bf16, tag=f"v{w}", name=f"v{w}")
                   for w in range(NWIN)]
            ytn = cst.tile([128, HLOC, S], bf16, tag="ytn")

            with tc.tile_pool(name="xs", bufs=2) as xs, \
                 tc.tile_pool(name="qtw", bufs=2) as qtw, \
                 tc.tile_pool(name="nat", bufs=3) as nat, \
                 tc.tile_pool(name="rms", bufs=2) as rms, \
                 tc.tile_pool(name="rop", bufs=2) as rop, \
                 tc.tile_pool(name="ex", bufs=4) as ex, \
                 tc.tile_pool(name="sm", bufs=2) as sm, \
                 tc.tile_pool(name="p1a", bufs=2, space="PSUM") as p1a, \
                 tc.tile_pool(name="p1t", bufs=1, space="PSUM") as p1t, \
                 tc.tile_pool(name="p2s", bufs=3, space="PSUM") as p2s, \
                 tc.tile_pool(name="p2y", bufs=2, space="PSUM") as p2y:

                qt_tiles = {}
                pending_tp = []

                def emit_one_transpose():
                    if pending_tp:
                        src, dst = pending_tp.pop(0)
                        tp = p1t.tile([128, 128], bf16, tag="tp", name="tp")
                        nc.tensor.transpose(tp[:], src, ident[:])
                        nc.vector.tensor_copy(dst, tp[:])

                def a_quanta(w):
                    """Generator of emission quanta for QKV production of
                    window w. Each quantum is a small chunk of PE work (or
                    the DVE/ACT rope tail of a tile)."""
                    qt_sb = qtw.tile([128, HLOC, 512], bf16, tag="qt",
                                     name=f"qt{w}")
                    qt_tiles[w] = qt_sb
                    for tt in range(4):
                        tcid = 4 * w + tt
                        xt_sb = xs.tile([128, NDT, 128], bf16, tag="xt",
                                        name=f"xt{tcid}")
                        nc.sync.dma_start(
                            out=xt_sb[:],
                            in_=xt_d[tcid].rearrange("p (t s) -> p t s",
                                                     t=NDT))
                        ps_q1 = p1a.tile([128, 512], f32, tag="acc", name="psq1")
                        ps_q2 = p1a.tile([128, 512], f32, tag="acc", name="psq2")
                        ps_kv = p1a.tile([128, 512], f32, tag="acc", name="pskv")

                        def triple(dt_, a=ps_q1, b=ps_q2, c=ps_kv, x=xt_sb):
                            st, sp = dt_ == 0, dt_ == NDT - 1
                            lhs = x[:, dt_, :]
                            nc.tensor.matmul(a[:], lhs, wq_sb[:, dt_, 0:512],
                                             start=st, stop=sp)
                            nc.tensor.matmul(b[:], lhs,
                                             wq_sb[:, dt_, 512:1024],
                                             start=st, stop=sp)
                            nc.tensor.matmul(c[:], lhs, wkv_sb[:, dt_, :],
                                             start=st, stop=sp)
                            emit_one_transpose()

                        for dt_ in range(NDT):
                            yield lambda d=dt_: triple(d)

                        def tile_tail(tt=tt, tcid=tcid, ps_q1=ps_q1,
                                      ps_q2=ps_q2, ps_kv=ps_kv, qt_sb=qt_sb):
                            cos1 = cos_sb[:, tcid:tcid + 1, :]
                            sin1 = sin_sb[:, tcid:tcid + 1, :]
                            ms = rms.tile([128, NH_ALL, 1], f32, tag="ms",
                                          name="ms")
                            groups = ((ps_q1, 4, 0), (ps_q2, 4, 4),
                                      (ps_kv, 2, 8))
                            t1s = []
                            sqs = []
                            for gi, (ps, nh, g0) in enumerate(groups):
                                wdt = nh * 128
                                shp = [128, nh, 2, 64]
                                p4 = ps[:, 0:wdt].rearrange(
                                    "p (h x f) -> p h x f", h=nh, x=2)
                                p4s = p4[:, :, ::-1, :]
                                cb = cos1.rearrange(
                                    "p t (x f) -> p t x f", x=2) \
                                    .to_broadcast(shp)
                                sb_ = sin1.rearrange(
                                    "p t (x f) -> p t x f", x=2) \
                                    .to_broadcast(shp)
                                sq = rop.tile([128, 512], bf16, tag="sq",
                                              bufs=2, name="sq")
                                t1 = rop.tile([128, 4, 2, 64], f32, tag="t1",
                                              bufs=3, name="t1")
                                t2 = rop.tile([128, 4, 2, 64], f32, tag="t2",
                                              bufs=2, name="t2")
                                nc.scalar.activation(sq[:, 0:wdt],
                                                     ps[:, 0:wdt], Act.Square)
                                nc.vector.tensor_mul(t1[:, 0:nh], p4, cb)
                                nc.vector.tensor_mul(t2[:, 0:nh], p4s, sb_)
                                nc.vector.tensor_add(t1[:, 0:nh], t1[:, 0:nh],
                                                     t2[:, 0:nh])
                                t1s.append((t1, nh, g0))
                                sqs.append((sq, nh, g0))
                            nc.scalar.activation(v_w[w][:, tt, :],
                                                 ps_kv[:, 256:512], Act.Copy)
                            for sq, nh, g0 in sqs:
                                nc.vector.tensor_reduce(
                                    ms[:, g0:g0 + nh, :],
                                    sq[:, 0:nh * 128].rearrange(
                                        "p (h f) -> p h f", h=nh),
                                    axis=mybir.AxisListType.X, op=Alu.add)
                            yi = rms.tile([128, NH_ALL], i32, tag="yi",
                                          name="yi")
                            rinv = rms.tile([128, NH_ALL, 1, 1], f32,
                                            tag="rinv", name="rinv")
                            tq = rms.tile([128, NH_ALL], f32, tag="tq",
                                          name="tq")
                            _emit_quake_rsqrt(
                                nc, ms[:].rearrange("p h f -> p (h f)"),
                                yi[:],
                                rinv[:].rearrange("p h x f -> p (h x f)"),
                                tq[:])
                            qn = nat.tile([128, 1024], bf16, tag="qn",
                                          name="qn")
                            kn = nat.tile([128, 256], bf16, tag="kn",
                                          name="kn")
                            outs = (qn[:, 0:512], qn[:, 512:1024],
                                    kn[:, 0:256])
                            for (t1, nh, g0), o in zip(t1s, outs):
                                shp = [128, nh, 2, 64]
                                rb = rinv[:, g0:g0 + nh].to_broadcast(shp)
                                nc.vector.tensor_mul(
                                    o.rearrange("p (h x f) -> p h x f",
                                                h=nh, x=2),
                                    t1[:, 0:nh], rb)
                            for h in range(HLOC):
                                pending_tp.append(
                                    (qn[:, h * 128:(h + 1) * 128],
                                     qt_sb[:, h, tt * 128:(tt + 1) * 128]))
                            for kh in range(KVLOC):
                                pending_tp.append(
                                    (kn[:, kh * 128:(kh + 1) * 128],
                                     kt_w[w][:, kh,
                                             tt * 128:(tt + 1) * 128]))

                        yield tile_tail
                    # flush any transposes not yet drained by triples
                    while pending_tp:
                        yield emit_one_transpose

                def c_quanta(windows):
                    """Generator of out-projection quanta: one PSUM group
                    (8 accumulating matmuls + copy + store) per quantum."""
                    for wv in windows:
                        for og in range(4):
                            for tt in range(4):
                                tcid = 4 * wv + tt

                                def c_group(og=og, tcid=tcid):
                                    ps_o = p1a.tile([128, 512], f32,
                                                    tag="acc", name="pso")
                                    for h in range(HLOC):
                                        nc.tensor.matmul(
                                            ps_o[:],
                                            ytn[:, h,
                                                tcid * 128:(tcid + 1) * 128],
                                            wo_sb[:, h,
                                                  og * 512:(og + 1) * 512],
                                            start=(h == 0),
                                            stop=(h == HLOC - 1))
                                    ot = ex.tile([128, 512], bf16, tag="ot",
                                                 bufs=2, name="ot")
                                    nc.scalar.activation(ot[:], ps_o[:],
                                                         Act.Copy)
                                    nc.scalar.dma_start(
                                        out=out_d[
                                            tcid * 128:(tcid + 1) * 128,
                                            og * 512:(og + 1) * 512],
                                        in_=ot[:])

                                yield c_group

                def drain(gen):
                    for q in gen:
                        q()

                # window 0 production runs solo
                drain(a_quanta(0))

                tail_pending = []

                def flush_tail():
                    while tail_pending:
                        tail_pending.pop(0)()

                for w in range(NWIN):
                    qt_sb = qt_tiles[w]
                    if w < NWIN - 1:
                        filler = a_quanta(w + 1)
                        fills = 4 * (NDT + 1) + 10
                    else:
                        filler = c_quanta([0, 1, 2])
                        fills = 48
                    njt = 4 * w + 4
                    steps = HLOC * njt
                    acc = 0.0
                    rate = fills / steps

                    for hq in range(HLOC):
                        kvh = hq // 4
                        ps_y = p2y.tile([128, 512], f32, tag="y", name="psy")
                        ps_s = p2s.tile([1, 512], f32, tag="sc", name="pss")
                        rhs_q = qt_sb[:, hq, :]
                        def emit_yv(j, et):
                            jw, jj = j // 4, j % 4
                            st, sp = j == 0, j == njt - 1
                            nc.tensor.matmul(
                                ps_y[:],
                                v_w[jw][:, jj, kvh * 128:(kvh + 1) * 128],
                                et[:], start=st, stop=sp,
                                skip_group_check=True)
                            nc.tensor.matmul(
                                ps_s[:], ones[:], et[:],
                                start=st, stop=sp, skip_group_check=True)

                        prev_et = None
                        for j in range(njt):
                            jw, jj = j // 4, j % 4
                            ps_sc = p2s.tile([128, 512], f32, tag="sc",
                                             name="pssc")
                            nc.tensor.matmul(
                                ps_sc[:],
                                kt_w[jw][:, kvh, jj * 128:(jj + 1) * 128],
                                rhs_q)
                            if j >= 4 * w:
                                nc.vector.tensor_add(ps_sc[:], ps_sc[:],
                                                     masks[:, j - 4 * w, :])
                            et = ex.tile([128, 512], bf16, tag="et",
                                         name="et")
                            nc.scalar.activation(et[:], ps_sc[:], Act.Exp,
                                                 scale=SCALE)
                            # deferred normalize tail of the previous head
                            if j == 1:
                                flush_tail()
                            acc += rate
                            while acc >= 1.0:
                                acc -= 1.0
                                q = next(filler, None)
                                if q is not None:
                                    q()
                            if prev_et is not None:
                                emit_yv(j - 1, prev_et)
                            prev_et = et
                        emit_yv(njt - 1, prev_et)

                        rec = sm.tile([1, 512], f32, tag="rec", name="rec")
                        nc.vector.reciprocal_approx_fast(out=rec[:],
                                                         in_=ps_s[:])

                        def norm_tail(hq=hq, ps_y=ps_y, rec=rec, w=w):
                            rec_r = sm.tile([1, 512], f32r, tag="rec_r",
                                            name="rec_r")
                            nc.vector.tensor_copy(rec_r[:], rec[:])
                            bcp = p2s.tile([128, 512], f32, tag="sc",
                                           name="bcp")
                            nc.tensor.matmul(bcp[:], ones_r[:], rec_r[:])
                            bc = sm.tile([128, 512], bf16, tag="bc",
                                         name="bc")
                            nc.scalar.activation(bc[:], bcp[:], Act.Copy)
                            nc.vector.tensor_mul(
                                ytn[:, hq, w * 512:(w + 1) * 512],
                                ps_y[:], bc[:])

                        tail_pending.append(norm_tail)

                    # end of window: drain remaining filler, then the last
                    # head's tail (covered by the drained matmuls)
                    drain(filler)
                    flush_tail()

                # remaining out-projection
                drain(c_quanta([3]))

    nc.compile()
    return nc


_PROGRAM = None


def _get_program():
    global _PROGRAM
    if _PROGRAM is None:
        _PROGRAM = build_program()
    return _PROGRAM


def make_in_maps(x, W_qkv, W_out):
    bf = ml_dtypes.bfloat16
    in_maps = []
    for c in range(8):
        b, t = c // 2, c % 2
        xtT = np.ascontiguousarray(x[b].T).astype(bf)        # [D, S]
        # pre-tile: xt[tc, p, dt*128 + s] = xtT[dt*128 + p, tc*128 + s]
        xt4 = xtT.reshape(NDT, 128, NTC, 128).transpose(2, 1, 0, 3)
        xt = np.ascontiguousarray(xt4).reshape(NTC, 128, D)
        wq = np.ascontiguousarray(W_qkv[:, t * 1024:(t + 1) * 1024]).astype(bf)
        wk = W_qkv[:, D + t * 256: D + (t + 1) * 256]
        wv = W_qkv[:, D + 512 + t * 256: D + 512 + (t + 1) * 256]
        wkv = np.ascontiguousarray(np.concatenate([wk, wv], axis=1)).astype(bf)
        wo = np.ascontiguousarray(W_out[t * 1024:(t + 1) * 1024, :]).astype(bf)
        in_maps.append({"xt": xt, "wq": wq, "wkv": wkv, "wo": wo})
    return in_maps


def kernel(x, W_qkv, W_out):
    from concourse.bass_utils import run_bass_kernel_spmd
    nc = _get_program()
    in_maps = make_in_maps(np.asarray(x, dtype=np.float32),
                           np.asarray(W_qkv, dtype=np.float32),
                           np.asarray(W_out, dtype=np.float32))
    res = run_bass_kernel_spmd(nc, in_maps, list(range(8)), trace=False)
    out = np.empty((B, S, D), dtype=np.float32)
    for b in range(B):
        out[b] = (res.results[2 * b]["out"].astype(np.float32)
                  + res.results[2 * b + 1]["out"].astype(np.float32))
    return out


# revision 19
# speedup vs baseline: 35.5898x; 1.0885x over previous
"""Causal self-attention (GQA + RMS-norm + RoPE) Trainium2 Bass kernel, v5.

Sharding: 8 cores = 4 batches x 2 head-groups. Core c = 2*b + t handles
batch b with Q heads [8t, 8t+8) and KV heads [2t, 2t+2). Each core computes
a partial output projection; the host upcasts + sums the two partials.

Design:
- bf16 operands everywhere (host-cast), fp32 PSUM accumulation.
- Fully SBUF-resident: no DRAM scratch spill of qT/kT/v.
- ACT runs only {Exp, Square, Copy} (one table set: zero table loads).
  RMS rsqrt via quake bit-trick + 2 Newton steps on DVE; softmax
  reciprocal via single-instruction reciprocal_approx_fast on DVE.
- Host pre-tiles x / cos / sin / masks so every DMA line is >=4KB
  per partition.
- Emission interleaves independent PE work into the attention j-loops
  (QKV production of window w+1, out-projection of finished windows)
  so the in-order PE queue never starves on mask/exp latency; per-head
  softmax-normalize tails are deferred one head for the same reason.
"""
import sys
sys.path.insert(0, '/opt/trn_rl_repo')
import numpy as np
import ml_dtypes

from concourse import bass, bacc, mybir, tile

f32 = mybir.dt.float32
f32r = mybir.dt.float32r
bf16 = mybir.dt.bfloat16
i32 = mybir.dt.int32
Alu = mybir.AluOpType
Act = mybir.ActivationFunctionType

B, S, D = 4, 2048, 2048
H, HKV, HD = 16, 4, 128
HLOC = H // 2          # 8 q heads per core
KVLOC = HKV // 2       # 2 kv heads per core
SCALE = float(HD) ** -0.5
ROPE_BASE = 10000.0
MAGIC = 0x5F3759DF
S128 = float(np.sqrt(128.0))

NTC = S // 128         # 16 token tiles
NDT = D // 128         # 16 contraction tiles
NWIN = S // 512        # 4 windows
NH_ALL = HLOC + KVLOC  # 10 rms'd heads per token tile


def _rope_tables():
    inv_freq = (1.0 / (ROPE_BASE ** (np.arange(0, HD, 2, dtype=np.float64) / HD)))
    freqs = np.arange(S, dtype=np.float64)[:, None] * inv_freq[None, :]
    cos = np.cos(freqs)
    sin = np.sin(freqs)
    cos2 = np.concatenate([cos, cos], axis=1).astype(ml_dtypes.bfloat16)
    sin2 = np.concatenate([sin, -sin], axis=1).astype(ml_dtypes.bfloat16)
    # partition-major pre-tiling: [p, tcid, f] with p = token % 128
    cosA = np.ascontiguousarray(cos2.reshape(NTC, 128, HD).transpose(1, 0, 2))
    sinA = np.ascontiguousarray(sin2.reshape(NTC, 128, HD).transpose(1, 0, 2))
    return cosA, sinA


def _tri_masks():
    # mask[p, vi, f] = -1e30 where kv > q for scoresT diag tiles:
    # kv = 128*j + p, q = 512*w + f, vi = j - 4*w -> masked iff p + 128*vi > f
    m = np.zeros((4, 128, 512), dtype=np.float32)
    p = np.arange(128)[:, None]
    f = np.arange(512)[None, :]
    for vi in range(4):
        m[vi][(p + 128 * vi) > f] = -1e30
    return np.ascontiguousarray(
        m.astype(ml_dtypes.bfloat16).transpose(1, 0, 2))


def _emit_quake_rsqrt(nc, ms_ap, yi_ap, yf_ap, t_ap):
    """yf = sqrt(128) * rsqrt(ms) elementwise on [128, n] fp32 APs."""
    nc.vector.tensor_single_scalar(yi_ap, ms_ap.bitcast(i32), 1,
                                   Alu.logical_shift_right)
    nc.vector.tensor_single_scalar(yi_ap, yi_ap, 0xFFFFFFFF, Alu.bitwise_xor)
    nc.vector.tensor_single_scalar(yi_ap, yi_ap, MAGIC + 1, Alu.add)
    nc.vector.tensor_copy(yf_ap, yi_ap.bitcast(f32))
    for last in (False, True):
        nc.vector.tensor_mul(t_ap, yf_ap, yf_ap)
        nc.vector.tensor_mul(t_ap, t_ap, ms_ap)
        s_ = S128 if last else 1.0
        nc.vector.tensor_scalar(t_ap, t_ap, -0.5 * s_, 1.5 * s_,
                                Alu.mult, Alu.add)
        nc.vector.tensor_mul(yf_ap, yf_ap, t_ap)


def build_program():
    cos_np, sin_np = _rope_tables()
    masks_np = _tri_masks()

    nc = bacc.Bacc(trn_type="TRN2")

    # xt pre-tiled on host: xt[tc, p, dt*128+s] = x.T[dt*128+p, tc*128+s]
    xt_d = nc.dram_tensor("xt", [NTC, 128, D], bf16, kind="ExternalInput")
    wq_d = nc.dram_tensor("wq", [D, HLOC * HD], bf16, kind="ExternalInput")
    wkv_d = nc.dram_tensor("wkv", [D, 2 * KVLOC * HD], bf16, kind="ExternalInput")
    wo_d = nc.dram_tensor("wo", [HLOC * HD, D], bf16, kind="ExternalInput")
    out_d = nc.dram_tensor("out", [S, D], bf16, kind="ExternalOutput")

    cos_d = nc.inline_tensor(cos_np, "cos_t")
    sin_d = nc.inline_tensor(sin_np, "sin_t")
    ident_d = nc.inline_tensor(
        np.eye(128, dtype=np.float32).astype(ml_dtypes.bfloat16), "ident")
    masks_d = nc.inline_tensor(masks_np, "tri_masks")
    onescol_d = nc.inline_tensor(np.ones((128, 1), dtype=ml_dtypes.bfloat16),
                                 "onescol")
    onesrow_d = nc.inline_tensor(np.ones((1, 128), dtype=np.float32), "onesrow")

    with tile.TileContext(nc) as tc:
        with tc.tile_pool(name="cst", bufs=1) as cst:
            # tiny constants first (sync queue)
            ident = cst.tile([128, 128], bf16, tag="ident")
            ones = cst.tile([128, 1], bf16, tag="ones")
            ones_r = cst.tile([1, 128], f32r, tag="ones_r")
            nc.sync.dma_start(out=ident[:], in_=ident_d[:])
            nc.sync.dma_start(out=ones[:], in_=onescol_d[:])
            nc.sync.dma_start(out=ones_r[:], in_=onesrow_d[:].bitcast(f32r))

            # bulk constants on the vector DMA queue (off the x-load path)
            cos_sb = cst.tile([128, NTC, 128], bf16, tag="cos")
            sin_sb = cst.tile([128, NTC, 128], bf16, tag="sin")
            masks = cst.tile([128, 4, 512], bf16, tag="masks")
            nc.gpsimd.dma_start(out=cos_sb[:], in_=cos_d[:])
            nc.gpsimd.dma_start(out=sin_sb[:], in_=sin_d[:])
            nc.gpsimd.dma_start(out=masks[:], in_=masks_d[:])

            # weights: wq/wkv needed first (scalar + vector queues); wo last
            wq_sb = cst.tile([128, NDT, HLOC * HD], bf16, tag="wq")
            wkv_sb = cst.tile([128, NDT, 512], bf16, tag="wkv")
            wo_sb = cst.tile([128, HLOC, D], bf16, tag="wo")
            wq_r = wq_d[:].rearrange("(t p) c -> p t c", p=128)
            wkv_r = wkv_d[:].rearrange("(t p) c -> p t c", p=128)
            wo_r = wo_d[:].rearrange("(h p) c -> p h c", p=128)
            for dt_ in range(NDT):
                nc.scalar.dma_start(out=wq_sb[:, dt_, :], in_=wq_r[:, dt_, :])
                nc.scalar.dma_start(out=wkv_sb[:, dt_, :],
                                    in_=wkv_r[:, dt_, :])
            for og in range(4):
                nc.gpsimd.dma_start(out=wo_sb[:, :, og * 512:(og + 1) * 512],
                                    in_=wo_r[:, :, og * 512:(og + 1) * 512])

            # per-window K/V residents + full attention output
            kt_w = [cst.tile([128, KVLOC, 512], bf16, tag=f"kt{w}",
                             name=f"kt{w}")
                    for w in range(NWIN)]
            v_w = [cst.tile([128, 4, 256], bf16, tag=f"v{w}", name=f"v{w}")
                   for w in range(NWIN)]
            ytn = cst.tile([128, HLOC, S], bf16, tag="ytn")

            with tc.tile_pool(name="xs", bufs=2) as xs, \
                 tc.tile_pool(name="qtw", bufs=2) as qtw, \
                 tc.tile_pool(name="nat", bufs=3) as nat, \
                 tc.tile_pool(name="rms", bufs=2) as rms, \
                 tc.tile_pool(name="rop", bufs=2) as rop, \
                 tc.tile_pool(name="ex", bufs=4) as ex, \
                 tc.tile_pool(name="sm", bufs=2) as sm, \
                 tc.tile_pool(name="p1a", bufs=2, space="PSUM") as p1a, \
                 tc.tile_pool(name="p1t", bufs=1, space="PSUM") as p1t, \
                 tc.tile_pool(name="p2s", bufs=3, space="PSUM") as p2s, \
                 tc.tile_pool(name="p2y", bufs=2, space="PSUM") as p2y:

                qt_tiles = {}
                pending_tp = []

                def emit_one_transpose():
                    if pending_tp:
                        src, dst = pending_tp.pop(0)
                        tp = p1t.tile([128, 128], bf16, tag="tp", name="tp")
                        nc.tensor.transpose(tp[:], src, ident[:])
                        nc.vector.tensor_copy(dst, tp[:])

                def a_quanta(w):
                    """Generator of emission quanta for QKV production of
                    window w. Each quantum is a small chunk of PE work (or
                    the DVE/ACT rope tail of a tile)."""
                    qt_sb = qtw.tile([128, HLOC, 512], bf16, tag="qt",
                                     name=f"qt{w}")
                    qt_tiles[w] = qt_sb
                    for tt in range(4):
                        tcid = 4 * w + tt
                        xt_sb = xs.tile([128, NDT, 128], bf16, tag="xt",
                                        name=f"xt{tcid}")
                        nc.sync.dma_start(
                            out=xt_sb[:],
                            in_=xt_d[tcid].rearrange("p (t s) -> p t s",
                                                     t=NDT))
                        ps_q1 = p1a.tile([128, 512], f32, tag="acc", name="psq1")
                        ps_q2 = p1a.tile([128, 512], f32, tag="acc", name="psq2")
                        ps_kv = p1a.tile([128, 512], f32, tag="acc", name="pskv")

                        def triple(dt_, a=ps_q1, b=ps_q2, c=ps_kv, x=xt_sb):
                            st, sp = dt_ == 0, dt_ == NDT - 1
                            lhs = x[:, dt_, :]
                            nc.tensor.matmul(a[:], lhs, wq_sb[:, dt_, 0:512],
                                             start=st, stop=sp)
                            nc.tensor.matmul(b[:], lhs,
                                             wq_sb[:, dt_, 512:1024],
                                             start=st, stop=sp)
                            nc.tensor.matmul(c[:], lhs, wkv_sb[:, dt_, :],
                                             start=st, stop=sp)
                            emit_one_transpose()

                        for dt_ in range(NDT):
                            yield lambda d=dt_: triple(d)

                        def tile_tail(tt=tt, tcid=tcid, ps_q1=ps_q1,
                                      ps_q2=ps_q2, ps_kv=ps_kv, qt_sb=qt_sb):
                            cos1 = cos_sb[:, tcid:tcid + 1, :]
                            sin1 = sin_sb[:, tcid:tcid + 1, :]
                            ms = rms.tile([128, NH_ALL, 1], f32, tag="ms",
                                          name="ms")
                            groups = ((ps_q1, 4, 0), (ps_q2, 4, 4),
                                      (ps_kv, 2, 8))
                            t1s = []
                            sqs = []
                            for gi, (ps, nh, g0) in enumerate(groups):
                                wdt = nh * 128
                                shp = [128, nh, 2, 64]
                                p4 = ps[:, 0:wdt].rearrange(
                                    "p (h x f) -> p h x f", h=nh, x=2)
                                p4s = p4[:, :, ::-1, :]
                                cb = cos1.rearrange(
                                    "p t (x f) -> p t x f", x=2) \
                                    .to_broadcast(shp)
                                sb_ = sin1.rearrange(
                                    "p t (x f) -> p t x f", x=2) \
                                    .to_broadcast(shp)
                                sq = rop.tile([128, 512], bf16, tag="sq",
                                              bufs=2, name="sq")
                                t1 = rop.tile([128, 4, 2, 64], f32, tag="t1",
                                              bufs=3, name="t1")
                                t2 = rop.tile([128, 4, 2, 64], f32, tag="t2",
                                              bufs=2, name="t2")
                                nc.scalar.activation(sq[:, 0:wdt],
                                                     ps[:, 0:wdt], Act.Square)
                                nc.vector.tensor_mul(t1[:, 0:nh], p4, cb)
                                nc.vector.tensor_mul(t2[:, 0:nh], p4s, sb_)
                                nc.vector.tensor_add(t1[:, 0:nh], t1[:, 0:nh],
                                                     t2[:, 0:nh])
                                t1s.append((t1, nh, g0))
                                sqs.append((sq, nh, g0))
                            nc.scalar.activation(v_w[w][:, tt, :],
                                                 ps_kv[:, 256:512], Act.Copy)
                            for sq, nh, g0 in sqs:
                                nc.vector.tensor_reduce(
                                    ms[:, g0:g0 + nh, :],
                                    sq[:, 0:nh * 128].rearrange(
                                        "p (h f) -> p h f", h=nh),
                                    axis=mybir.AxisListType.X, op=Alu.add)
                            yi = rms.tile([128, NH_ALL], i32, tag="yi",
                                          name="yi")
                            rinv = rms.tile([128, NH_ALL, 1, 1], f32,
                                            tag="rinv", name="rinv")
                            tq = rms.tile([128, NH_ALL], f32, tag="tq",
                                          name="tq")
                            _emit_quake_rsqrt(
                                nc, ms[:].rearrange("p h f -> p (h f)"),
                                yi[:],
                                rinv[:].rearrange("p h x f -> p (h x f)"),
                                tq[:])
                            qn = nat.tile([128, 1024], bf16, tag="qn",
                                          name="qn")
                            kn = nat.tile([128, 256], bf16, tag="kn",
                                          name="kn")
                            outs = (qn[:, 0:512], qn[:, 512:1024],
                                    kn[:, 0:256])
                            for (t1, nh, g0), o in zip(t1s, outs):
                                shp = [128, nh, 2, 64]
                                rb = rinv[:, g0:g0 + nh].to_broadcast(shp)
                                nc.vector.tensor_mul(
                                    o.rearrange("p (h x f) -> p h x f",
                                                h=nh, x=2),
                                    t1[:, 0:nh], rb)
                            for h in range(HLOC):
                                pending_tp.append(
                                    (qn[:, h * 128:(h + 1) * 128],
                                     qt_sb[:, h, tt * 128:(tt + 1) * 128]))
                            for kh in range(KVLOC):
                                pending_tp.append(
                                    (kn[:, kh * 128:(kh + 1) * 128],
                                     kt_w[w][:, kh,
                                             tt * 128:(tt + 1) * 128]))

                        yield tile_tail
                    # flush any transposes not yet drained by triples
                    while pending_tp:
                        yield emit_one_transpose

                def c_quanta(windows):
                    """Generator of out-projection quanta: one PSUM group
                    (8 accumulating matmuls + copy + store) per quantum."""
                    for wv in windows:
                        for og in range(4):
                            for tt in range(4):
                                tcid = 4 * wv + tt

                                def c_group(og=og, tcid=tcid):
                                    ps_o = p1a.tile([128, 512], f32,
                                                    tag="acc", name="pso")
                                    for h in range(HLOC):
                                        nc.tensor.matmul(
                                            ps_o[:],
                                            ytn[:, h,
                                                tcid * 128:(tcid + 1) * 128],
                                            wo_sb[:, h,
                                                  og * 512:(og + 1) * 512],
                                            start=(h == 0),
                                            stop=(h == HLOC - 1))
                                    ot = ex.tile([128, 512], bf16, tag="ot",
                                                 bufs=2, name="ot")
                                    nc.scalar.activation(ot[:], ps_o[:],
                                                         Act.Copy)
                                    nc.scalar.dma_start(
                                        out=out_d[
                                            tcid * 128:(tcid + 1) * 128,
                                            og * 512:(og + 1) * 512],
                                        in_=ot[:])

                                yield c_group

                def drain(gen):
                    for q in gen:
                        q()

                # window 0 production runs solo
                drain(a_quanta(0))

                tail_pending = []

                def flush_tail():
                    while tail_pending:
                        tail_pending.pop(0)()

                for w in range(NWIN):
                    qt_sb = qt_tiles[w]
                    if w < NWIN - 1:
                        filler = a_quanta(w + 1)
                        fills = 4 * (NDT + 1) + 10
                    else:
                        filler = c_quanta([0, 1, 2])
                        fills = 48
                    njt = 4 * w + 4
                    steps = HLOC * njt
                    acc = 0.0
                    rate = fills / steps

                    for hq in range(HLOC):
                        kvh = hq // 4
                        ps_y = p2y.tile([128, 512], f32, tag="y", name="psy")
                        s_acc = sm.tile([128, 512], bf16, tag="sacc",
                                        name="sacc")
                        ets = []
                        rhs_q = qt_sb[:, hq, :]
                        def emit_yv(j, et):
                            jw, jj = j // 4, j % 4
                            st, sp = j == 0, j == njt - 1
                            nc.tensor.matmul(
                                ps_y[:],
                                v_w[jw][:, jj, kvh * 128:(kvh + 1) * 128],
                                et[:], start=st, stop=sp,
                                skip_group_check=True)
                            if j == 1:
                                nc.vector.tensor_add(s_acc[:], ets[0][:],
                                                     ets[1][:])
                            elif j > 1:
                                nc.vector.tensor_add(s_acc[:], s_acc[:],
                                                     et[:])

                        prev_et = None
                        for j in range(njt):
                            jw, jj = j // 4, j % 4
                            ps_sc = p2s.tile([128, 512], f32, tag="sc",
                                             name="pssc")
                            nc.tensor.matmul(
                                ps_sc[:],
                                kt_w[jw][:, kvh, jj * 128:(jj + 1) * 128],
                                rhs_q)
                            if j >= 4 * w:
                                nc.vector.tensor_add(ps_sc[:], ps_sc[:],
                                                     masks[:, j - 4 * w, :])
                            et = ex.tile([128, 512], bf16, tag="et",
                                         name="et")
                            nc.scalar.activation(et[:], ps_sc[:], Act.Exp,
                                                 scale=SCALE)
                            ets.append(et)
                            # deferred normalize tail of the previous head
                            if j == 1:
                                flush_tail()
                            acc += rate
                            while acc >= 1.0:
                                acc -= 1.0
                                q = next(filler, None)
                                if q is not None:
                                    q()
                            if prev_et is not None:
                                emit_yv(j - 1, prev_et)
                            prev_et = et
                        emit_yv(njt - 1, prev_et)
                        ps_s = p2s.tile([1, 512], f32, tag="sc", name="pss")
                        nc.tensor.matmul(ps_s[:], ones[:], s_acc[:])
                        rec = sm.tile([1, 512], f32, tag="rec", name="rec")
                        nc.vector.reciprocal_approx_fast(out=rec[:],
                                                         in_=ps_s[:])

                        def norm_tail(hq=hq, ps_y=ps_y, rec=rec, w=w):
                            rec_r = sm.tile([1, 512], f32r, tag="rec_r",
                                            name="rec_r")
                            nc.vector.tensor_copy(rec_r[:], rec[:])
                            bcp = p2s.tile([128, 512], f32, tag="sc",
                                           name="bcp")
                            nc.tensor.matmul(bcp[:], ones_r[:], rec_r[:])
                            bc = sm.tile([128, 512], bf16, tag="bc",
                                         name="bc")
                            nc.scalar.activation(bc[:], bcp[:], Act.Copy)
                            nc.vector.tensor_mul(
                                ytn[:, hq, w * 512:(w + 1) * 512],
                                ps_y[:], bc[:])

                        tail_pending.append(norm_tail)

                    # end of window: drain remaining filler, then the last
                    # head's tail (covered by the drained matmuls)
                    drain(filler)
                    flush_tail()

                # remaining out-projection
                drain(c_quanta([3]))

    nc.compile()
    return nc


_PROGRAM = None


def _get_program():
    global _PROGRAM
    if _PROGRAM is None:
        _PROGRAM = build_program()
    return _PROGRAM


def make_in_maps(x, W_qkv, W_out):
    bf = ml_dtypes.bfloat16
    in_maps = []
    for c in range(8):
        b, t = c // 2, c % 2
        xtT = np.ascontiguousarray(x[b].T).astype(bf)        # [D, S]
        # pre-tile: xt[tc, p, dt*128 + s] = xtT[dt*128 + p, tc*128 + s]
        xt4 = xtT.reshape(NDT, 128, NTC, 128).transpose(2, 1, 0, 3)
        xt = np.ascontiguousarray(xt4).reshape(NTC, 128, D)
        wq = np.ascontiguousarray(W_qkv[:, t * 1024:(t + 1) * 1024]).astype(bf)
        wk = W_qkv[:, D + t * 256: D + (t + 1) * 256]
        wv = W_qkv[:, D + 512 + t * 256: D + 512 + (t + 1) * 256]
        wkv = np.ascontiguousarray(np.concatenate([wk, wv], axis=1)).astype(bf)
        wo = np.ascontiguousarray(W_out[t * 1024:(t + 1) * 1024, :]).astype(bf)
        in_maps.append({"xt": xt, "wq": wq, "wkv": wkv, "wo": wo})
    return in_maps


def kernel(x, W_qkv, W_out):
    from concourse.bass_utils import run_bass_kernel_spmd
    nc = _get_program()
    in_maps = make_in_maps(np.asarray(x, dtype=np.float32),
                           np.asarray(W_qkv, dtype=np.float32),
                           np.asarray(W_out, dtype=np.float32))
    res = run_bass_kernel_spmd(nc, in_maps, list(range(8)), trace=False)
    out = np.empty((B, S, D), dtype=np.float32)
    for b in range(B):
        out[b] = (res.results[2 * b]["out"].astype(np.float32)
                  + res.results[2 * b + 1]["out"].astype(np.float32))
    return out


# revision 20
# speedup vs baseline: 35.8648x; 1.0077x over previous
"""Causal self-attention (GQA + RMS-norm + RoPE) Trainium2 Bass kernel, v5.

Sharding: 8 cores = 4 batches x 2 head-groups. Core c = 2*b + t handles
batch b with Q heads [8t, 8t+8) and KV heads [2t, 2t+2). Each core computes
a partial output projection; the host upcasts + sums the two partials.

Design:
- bf16 operands everywhere (host-cast), fp32 PSUM accumulation.
- Fully SBUF-resident: no DRAM scratch spill of qT/kT/v.
- ACT runs only {Exp, Square, Copy} (one table set: zero table loads).
  RMS rsqrt via quake bit-trick + 2 Newton steps on DVE; softmax
  reciprocal via single-instruction reciprocal_approx_fast on DVE.
- Host pre-tiles x / cos / sin / masks so every DMA line is >=4KB
  per partition.
- Emission interleaves independent PE work into the attention j-loops
  (QKV production of window w+1, out-projection of finished windows)
  so the in-order PE queue never starves on mask/exp latency; per-head
  softmax-normalize tails are deferred one head for the same reason.
"""
import sys
sys.path.insert(0, '/opt/trn_rl_repo')
import numpy as np
import ml_dtypes

from concourse import bass, bacc, mybir, tile

f32 = mybir.dt.float32
f32r = mybir.dt.float32r
bf16 = mybir.dt.bfloat16
i32 = mybir.dt.int32
Alu = mybir.AluOpType
Act = mybir.ActivationFunctionType

B, S, D = 4, 2048, 2048
H, HKV, HD = 16, 4, 128
HLOC = H // 2          # 8 q heads per core
KVLOC = HKV // 2       # 2 kv heads per core
SCALE = float(HD) ** -0.5
ROPE_BASE = 10000.0
MAGIC = 0x5F3759DF
S128 = float(np.sqrt(128.0))

NTC = S // 128         # 16 token tiles
NDT = D // 128         # 16 contraction tiles
NWIN = S // 512        # 4 windows
NH_ALL = HLOC + KVLOC  # 10 rms'd heads per token tile


def _rope_tables():
    inv_freq = (1.0 / (ROPE_BASE ** (np.arange(0, HD, 2, dtype=np.float64) / HD)))
    freqs = np.arange(S, dtype=np.float64)[:, None] * inv_freq[None, :]
    cos = np.cos(freqs)
    sin = np.sin(freqs)
    cos2 = np.concatenate([cos, cos], axis=1).astype(ml_dtypes.bfloat16)
    sin2 = np.concatenate([sin, -sin], axis=1).astype(ml_dtypes.bfloat16)
    # partition-major pre-tiling: [p, tcid, f] with p = token % 128
    cosA = np.ascontiguousarray(cos2.reshape(NTC, 128, HD).transpose(1, 0, 2))
    sinA = np.ascontiguousarray(sin2.reshape(NTC, 128, HD).transpose(1, 0, 2))
    return cosA, sinA


def _tri_masks():
    # mask[p, vi, f] = -1e30 where kv > q for scoresT diag tiles:
    # kv = 128*j + p, q = 512*w + f, vi = j - 4*w -> masked iff p + 128*vi > f
    m = np.zeros((4, 128, 512), dtype=np.float32)
    p = np.arange(128)[:, None]
    f = np.arange(512)[None, :]
    for vi in range(4):
        m[vi][(p + 128 * vi) > f] = -1e30
    return np.ascontiguousarray(
        m.astype(ml_dtypes.bfloat16).transpose(1, 0, 2))


def _emit_quake_rsqrt(nc, ms_ap, yi_ap, yf_ap, t_ap):
    """yf = sqrt(128) * rsqrt(ms) elementwise on [128, n] fp32 APs."""
    nc.vector.tensor_single_scalar(yi_ap, ms_ap.bitcast(i32), 1,
                                   Alu.logical_shift_right)
    nc.vector.tensor_single_scalar(yi_ap, yi_ap, 0xFFFFFFFF, Alu.bitwise_xor)
    nc.vector.tensor_single_scalar(yi_ap, yi_ap, MAGIC + 1, Alu.add)
    nc.vector.tensor_copy(yf_ap, yi_ap.bitcast(f32))
    for last in (False, True):
        nc.vector.tensor_mul(t_ap, yf_ap, yf_ap)
        nc.vector.tensor_mul(t_ap, t_ap, ms_ap)
        s_ = S128 if last else 1.0
        nc.vector.tensor_scalar(t_ap, t_ap, -0.5 * s_, 1.5 * s_,
                                Alu.mult, Alu.add)
        nc.vector.tensor_mul(yf_ap, yf_ap, t_ap)


def build_program():
    cos_np, sin_np = _rope_tables()
    masks_np = _tri_masks()

    nc = bacc.Bacc(trn_type="TRN2")

    # xt pre-tiled on host: xt[tc, p, dt*128+s] = x.T[dt*128+p, tc*128+s]
    xt_d = nc.dram_tensor("xt", [NTC, 128, D], bf16, kind="ExternalInput")
    wq_d = nc.dram_tensor("wq", [D, HLOC * HD], bf16, kind="ExternalInput")
    wkv_d = nc.dram_tensor("wkv", [D, 2 * KVLOC * HD], bf16, kind="ExternalInput")
    wo_d = nc.dram_tensor("wo", [HLOC * HD, D], bf16, kind="ExternalInput")
    out_d = nc.dram_tensor("out", [S, D], bf16, kind="ExternalOutput")

    cos_d = nc.inline_tensor(cos_np, "cos_t")
    sin_d = nc.inline_tensor(sin_np, "sin_t")
    ident_d = nc.inline_tensor(
        np.eye(128, dtype=np.float32).astype(ml_dtypes.bfloat16), "ident")
    masks_d = nc.inline_tensor(masks_np, "tri_masks")
    onescol_d = nc.inline_tensor(np.ones((128, 1), dtype=ml_dtypes.bfloat16),
                                 "onescol")
    onesrow_d = nc.inline_tensor(np.ones((1, 128), dtype=np.float32), "onesrow")

    with tile.TileContext(nc) as tc:
        with tc.tile_pool(name="cst", bufs=1) as cst:
            # tiny constants first (sync queue)
            ident = cst.tile([128, 128], bf16, tag="ident")
            ones = cst.tile([128, 1], bf16, tag="ones")
            ones_r = cst.tile([1, 128], f32r, tag="ones_r")
            nc.sync.dma_start(out=ident[:], in_=ident_d[:])
            nc.sync.dma_start(out=ones[:], in_=onescol_d[:])
            nc.sync.dma_start(out=ones_r[:], in_=onesrow_d[:].bitcast(f32r))

            # weights first: wq on scalar queue, wkv on gpsimd queue
            wq_sb = cst.tile([128, NDT, HLOC * HD], bf16, tag="wq")
            wkv_sb = cst.tile([128, NDT, 512], bf16, tag="wkv")
            wo_sb = cst.tile([128, HLOC, D], bf16, tag="wo")
            wq_r = wq_d[:].rearrange("(t p) c -> p t c", p=128)
            wkv_r = wkv_d[:].rearrange("(t p) c -> p t c", p=128)
            wo_r = wo_d[:].rearrange("(h p) c -> p h c", p=128)
            for dt_ in range(NDT):
                nc.scalar.dma_start(out=wq_sb[:, dt_, :], in_=wq_r[:, dt_, :])
                nc.gpsimd.dma_start(out=wkv_sb[:, dt_, :],
                                    in_=wkv_r[:, dt_, :])

            # bulk constants behind wkv on the gpsimd queue; wo last on scalar
            cos_sb = cst.tile([128, NTC, 128], bf16, tag="cos")
            sin_sb = cst.tile([128, NTC, 128], bf16, tag="sin")
            masks = cst.tile([128, 4, 512], bf16, tag="masks")
            nc.gpsimd.dma_start(out=cos_sb[:], in_=cos_d[:])
            nc.gpsimd.dma_start(out=sin_sb[:], in_=sin_d[:])
            nc.gpsimd.dma_start(out=masks[:], in_=masks_d[:])
            for og in range(4):
                nc.scalar.dma_start(out=wo_sb[:, :, og * 512:(og + 1) * 512],
                                    in_=wo_r[:, :, og * 512:(og + 1) * 512])

            # per-window K/V residents + full attention output
            kt_w = [cst.tile([128, KVLOC, 512], bf16, tag=f"kt{w}",
                             name=f"kt{w}")
                    for w in range(NWIN)]
            v_w = [cst.tile([128, 4, 256], bf16, tag=f"v{w}", name=f"v{w}")
                   for w in range(NWIN)]
            ytn = cst.tile([128, HLOC, S], bf16, tag="ytn")

            with tc.tile_pool(name="xs", bufs=2) as xs, \
                 tc.tile_pool(name="qtw", bufs=2) as qtw, \
                 tc.tile_pool(name="nat", bufs=3) as nat, \
                 tc.tile_pool(name="rms", bufs=2) as rms, \
                 tc.tile_pool(name="rop", bufs=2) as rop, \
                 tc.tile_pool(name="ex", bufs=4) as ex, \
                 tc.tile_pool(name="sm", bufs=2) as sm, \
                 tc.tile_pool(name="p1a", bufs=2, space="PSUM") as p1a, \
                 tc.tile_pool(name="p1t", bufs=1, space="PSUM") as p1t, \
                 tc.tile_pool(name="p2s", bufs=3, space="PSUM") as p2s, \
                 tc.tile_pool(name="p2y", bufs=2, space="PSUM") as p2y:

                qt_tiles = {}
                pending_tp = []

                def emit_one_transpose():
                    if pending_tp:
                        src, dst = pending_tp.pop(0)
                        tp = p1t.tile([128, 128], bf16, tag="tp", name="tp")
                        nc.tensor.transpose(tp[:], src, ident[:])
                        nc.vector.tensor_copy(dst, tp[:])

                def a_quanta(w):
                    """Generator of emission quanta for QKV production of
                    window w. Each quantum is a small chunk of PE work (or
                    the DVE/ACT rope tail of a tile)."""
                    qt_sb = qtw.tile([128, HLOC, 512], bf16, tag="qt",
                                     name=f"qt{w}")
                    qt_tiles[w] = qt_sb
                    for tt in range(4):
                        tcid = 4 * w + tt
                        xt_sb = xs.tile([128, NDT, 128], bf16, tag="xt",
                                        name=f"xt{tcid}")
                        nc.sync.dma_start(
                            out=xt_sb[:],
                            in_=xt_d[tcid].rearrange("p (t s) -> p t s",
                                                     t=NDT))
                        ps_q1 = p1a.tile([128, 512], f32, tag="acc", name="psq1")
                        ps_q2 = p1a.tile([128, 512], f32, tag="acc", name="psq2")
                        ps_kv = p1a.tile([128, 512], f32, tag="acc", name="pskv")

                        def triple(dt_, a=ps_q1, b=ps_q2, c=ps_kv, x=xt_sb):
                            st, sp = dt_ == 0, dt_ == NDT - 1
                            lhs = x[:, dt_, :]
                            nc.tensor.matmul(a[:], lhs, wq_sb[:, dt_, 0:512],
                                             start=st, stop=sp)
                            nc.tensor.matmul(b[:], lhs,
                                             wq_sb[:, dt_, 512:1024],
                                             start=st, stop=sp)
                            nc.tensor.matmul(c[:], lhs, wkv_sb[:, dt_, :],
                                             start=st, stop=sp)
                            emit_one_transpose()

                        for dt_ in range(NDT):
                            yield lambda d=dt_: triple(d)

                        def tile_tail(tt=tt, tcid=tcid, ps_q1=ps_q1,
                                      ps_q2=ps_q2, ps_kv=ps_kv, qt_sb=qt_sb):
                            cos1 = cos_sb[:, tcid:tcid + 1, :]
                            sin1 = sin_sb[:, tcid:tcid + 1, :]
                            ms = rms.tile([128, NH_ALL, 1], f32, tag="ms",
                                          name="ms")
                            groups = ((ps_q1, 4, 0), (ps_q2, 4, 4),
                                      (ps_kv, 2, 8))
                            t1s = []
                            sqs = []
                            for gi, (ps, nh, g0) in enumerate(groups):
                                wdt = nh * 128
                                shp = [128, nh, 2, 64]
                                p4 = ps[:, 0:wdt].rearrange(
                                    "p (h x f) -> p h x f", h=nh, x=2)
                                p4s = p4[:, :, ::-1, :]
                                cb = cos1.rearrange(
                                    "p t (x f) -> p t x f", x=2) \
                                    .to_broadcast(shp)
                                sb_ = sin1.rearrange(
                                    "p t (x f) -> p t x f", x=2) \
                                    .to_broadcast(shp)
                                sq = rop.tile([128, 512], bf16, tag="sq",
                                              bufs=2, name="sq")
                                t1 = rop.tile([128, 4, 2, 64], f32, tag="t1",
                                              bufs=3, name="t1")
                                t2 = rop.tile([128, 4, 2, 64], f32, tag="t2",
                                              bufs=2, name="t2")
                                nc.scalar.activation(sq[:, 0:wdt],
                                                     ps[:, 0:wdt], Act.Square)
                                nc.vector.tensor_mul(t1[:, 0:nh], p4, cb)
                                nc.vector.tensor_mul(t2[:, 0:nh], p4s, sb_)
                                nc.vector.tensor_add(t1[:, 0:nh], t1[:, 0:nh],
                                                     t2[:, 0:nh])
                                t1s.append((t1, nh, g0))
                                sqs.append((sq, nh, g0))
                            nc.scalar.activation(v_w[w][:, tt, :],
                                                 ps_kv[:, 256:512], Act.Copy)
                            for sq, nh, g0 in sqs:
                                nc.vector.tensor_reduce(
                                    ms[:, g0:g0 + nh, :],
                                    sq[:, 0:nh * 128].rearrange(
                                        "p (h f) -> p h f", h=nh),
                                    axis=mybir.AxisListType.X, op=Alu.add)
                            yi = rms.tile([128, NH_ALL], i32, tag="yi",
                                          name="yi")
                            rinv = rms.tile([128, NH_ALL, 1, 1], f32,
                                            tag="rinv", name="rinv")
                            tq = rms.tile([128, NH_ALL], f32, tag="tq",
                                          name="tq")
                            _emit_quake_rsqrt(
                                nc, ms[:].rearrange("p h f -> p (h f)"),
                                yi[:],
                                rinv[:].rearrange("p h x f -> p (h x f)"),
                                tq[:])
                            qn = nat.tile([128, 1024], bf16, tag="qn",
                                          name="qn")
                            kn = nat.tile([128, 256], bf16, tag="kn",
                                          name="kn")
                            outs = (qn[:, 0:512], qn[:, 512:1024],
                                    kn[:, 0:256])
                            for (t1, nh, g0), o in zip(t1s, outs):
                                shp = [128, nh, 2, 64]
                                rb = rinv[:, g0:g0 + nh].to_broadcast(shp)
                                nc.vector.tensor_mul(
                                    o.rearrange("p (h x f) -> p h x f",
                                                h=nh, x=2),
                                    t1[:, 0:nh], rb)
                            for h in range(HLOC):
                                pending_tp.append(
                                    (qn[:, h * 128:(h + 1) * 128],
                                     qt_sb[:, h, tt * 128:(tt + 1) * 128]))
                            for kh in range(KVLOC):
                                pending_tp.append(
                                    (kn[:, kh * 128:(kh + 1) * 128],
                                     kt_w[w][:, kh,
                                             tt * 128:(tt + 1) * 128]))

                        yield tile_tail
                    # flush any transposes not yet drained by triples
                    while pending_tp:
                        yield emit_one_transpose

                def c_quanta(windows):
                    """Generator of out-projection quanta: one PSUM group
                    (8 accumulating matmuls + copy + store) per quantum."""
                    for wv in windows:
                        for og in range(4):
                            for tt in range(4):
                                tcid = 4 * wv + tt

                                def c_group(og=og, tcid=tcid):
                                    ps_o = p1a.tile([128, 512], f32,
                                                    tag="acc", name="pso")
                                    for h in range(HLOC):
                                        nc.tensor.matmul(
                                            ps_o[:],
                                            ytn[:, h,
                                                tcid * 128:(tcid + 1) * 128],
                                            wo_sb[:, h,
                                                  og * 512:(og + 1) * 512],
                                            start=(h == 0),
                                            stop=(h == HLOC - 1))
                                    ot = ex.tile([128, 512], bf16, tag="ot",
                                                 bufs=2, name="ot")
                                    nc.scalar.activation(ot[:], ps_o[:],
                                                         Act.Copy)
                                    nc.scalar.dma_start(
                                        out=out_d[
                                            tcid * 128:(tcid + 1) * 128,
                                            og * 512:(og + 1) * 512],
                                        in_=ot[:])

                                yield c_group

                def drain(gen):
                    for q in gen:
                        q()

                # window 0 production runs solo
                drain(a_quanta(0))

                tail_pending = []

                def flush_tail():
                    while tail_pending:
                        tail_pending.pop(0)()

                for w in range(NWIN):
                    qt_sb = qt_tiles[w]
                    if w < NWIN - 1:
                        filler = a_quanta(w + 1)
                        fills = 4 * (NDT + 1) + 10
                    else:
                        filler = c_quanta([0, 1, 2])
                        fills = 48
                    njt = 4 * w + 4
                    steps = HLOC * njt
                    acc = 0.0
                    rate = fills / steps

                    for hq in range(HLOC):
                        kvh = hq // 4
                        ps_y = p2y.tile([128, 512], f32, tag="y", name="psy")
                        s_acc = sm.tile([128, 512], bf16, tag="sacc",
                                        name="sacc")
                        ets = []
                        rhs_q = qt_sb[:, hq, :]
                        def emit_yv(j, et):
                            jw, jj = j // 4, j % 4
                            st, sp = j == 0, j == njt - 1
                            nc.tensor.matmul(
                                ps_y[:],
                                v_w[jw][:, jj, kvh * 128:(kvh + 1) * 128],
                                et[:], start=st, stop=sp,
                                skip_group_check=True)
                            if j == 1:
                                nc.vector.tensor_add(s_acc[:], ets[0][:],
                                                     ets[1][:])
                            elif j > 1:
                                nc.vector.tensor_add(s_acc[:], s_acc[:],
                                                     et[:])

                        prev_et = None
                        for j in range(njt):
                            jw, jj = j // 4, j % 4
                            ps_sc = p2s.tile([128, 512], f32, tag="sc",
                                             name="pssc")
                            nc.tensor.matmul(
                                ps_sc[:],
                                kt_w[jw][:, kvh, jj * 128:(jj + 1) * 128],
                                rhs_q)
                            if j >= 4 * w:
                                nc.vector.tensor_add(ps_sc[:], ps_sc[:],
                                                     masks[:, j - 4 * w, :])
                            et = ex.tile([128, 512], bf16, tag="et",
                                         name="et")
                            nc.scalar.activation(et[:], ps_sc[:], Act.Exp,
                                                 scale=SCALE)
                            ets.append(et)
                            # deferred normalize tail of the previous head
                            if j == 1:
                                flush_tail()
                            acc += rate
                            while acc >= 1.0:
                                acc -= 1.0
                                q = next(filler, None)
                                if q is not None:
                                    q()
                            if prev_et is not None:
                                emit_yv(j - 1, prev_et)
                            prev_et = et
                        emit_yv(njt - 1, prev_et)
                        ps_s = p2s.tile([1, 512], f32, tag="sc", name="pss")
                        nc.tensor.matmul(ps_s[:], ones[:], s_acc[:])
                        rec = sm.tile([1, 512], f32, tag="rec", name="rec")
                        nc.vector.reciprocal_approx_fast(out=rec[:],
                                                         in_=ps_s[:])

                        def norm_tail(hq=hq, ps_y=ps_y, rec=rec, w=w):
                            rec_r = sm.tile([1, 512], f32r, tag="rec_r",
                                            name="rec_r")
                            nc.vector.tensor_copy(rec_r[:], rec[:])
                            bcp = p2s.tile([128, 512], f32, tag="sc",
                                           name="bcp")
                            nc.tensor.matmul(bcp[:], ones_r[:], rec_r[:])
                            bc = sm.tile([128, 512], bf16, tag="bc",
                                         name="bc")
                            nc.scalar.activation(bc[:], bcp[:], Act.Copy)
                            nc.vector.tensor_mul(
                                ytn[:, hq, w * 512:(w + 1) * 512],
                                ps_y[:], bc[:])

                        tail_pending.append(norm_tail)

                    # end of window: drain remaining filler, then the last
                    # head's tail (covered by the drained matmuls)
                    drain(filler)
                    flush_tail()

                # remaining out-projection
                drain(c_quanta([3]))

    nc.compile()
    return nc


_PROGRAM = None


def _get_program():
    global _PROGRAM
    if _PROGRAM is None:
        _PROGRAM = build_program()
    return _PROGRAM


def make_in_maps(x, W_qkv, W_out):
    bf = ml_dtypes.bfloat16
    in_maps = []
    for c in range(8):
        b, t = c // 2, c % 2
        xtT = np.ascontiguousarray(x[b].T).astype(bf)        # [D, S]
        # pre-tile: xt[tc, p, dt*128 + s] = xtT[dt*128 + p, tc*128 + s]
        xt4 = xtT.reshape(NDT, 128, NTC, 128).transpose(2, 1, 0, 3)
        xt = np.ascontiguousarray(xt4).reshape(NTC, 128, D)
        wq = np.ascontiguousarray(W_qkv[:, t * 1024:(t + 1) * 1024]).astype(bf)
        wk = W_qkv[:, D + t * 256: D + (t + 1) * 256]
        wv = W_qkv[:, D + 512 + t * 256: D + 512 + (t + 1) * 256]
        wkv = np.ascontiguousarray(np.concatenate([wk, wv], axis=1)).astype(bf)
        wo = np.ascontiguousarray(W_out[t * 1024:(t + 1) * 1024, :]).astype(bf)
        in_maps.append({"xt": xt, "wq": wq, "wkv": wkv, "wo": wo})
    return in_maps


def kernel(x, W_qkv, W_out):
    from concourse.bass_utils import run_bass_kernel_spmd
    nc = _get_program()
    in_maps = make_in_maps(np.asarray(x, dtype=np.float32),
                           np.asarray(W_qkv, dtype=np.float32),
                           np.asarray(W_out, dtype=np.float32))
    res = run_bass_kernel_spmd(nc, in_maps, list(range(8)), trace=False)
    out = np.empty((B, S, D), dtype=np.float32)
    for b in range(B):
        out[b] = (res.results[2 * b]["out"].astype(np.float32)
                  + res.results[2 * b + 1]["out"].astype(np.float32))
    return out
